# revision 1
# baseline (speedup 1.0000x reference)
"""Trainium2 Bass kernel for linear attention (ELU+1 feature map) block:
Q/K/V projections + linear attention + out-projection + residual + LayerNorm,
distributed over 8 NeuronCores.

Sharding: 8-way row split of the (batch*seq) dimension. Cores 2b and 2b+1
process the two 2048-row halves of batch b; the per-(batch,head) global
reductions KtQ [H,64,64] and q_sum [D] are pair-AllReduced on device.
"""
import os
import sys

for _p in ("/opt/trn_rl_repo", "/root/.axon_site/_ro/trn_rl_repo"):
    if os.path.isdir(_p) and _p not in sys.path:
        sys.path.insert(0, _p)

import numpy as np

B, N, D, H = 4, 4096, 1024, 16
DEPTH = D // H  # 64
NCORES = 8
R = (B * N) // NCORES  # 2048 rows per core
NSUB = R // 128  # 16 token subtiles per core
NBLK = R // 512  # 4 token blocks per core
EPS_Z = 1e-9
EPS_LN = 1e-6

_CACHE = {}


def _build(zb_qk, zb_v, zb_o, g_one, b_zero, single=False):
    import concourse.bacc as bacc
    import concourse.tile as tile
    from concourse import mybir
    from concourse.masks import make_identity
    from contextlib import ExitStack

    F32 = mybir.dt.float32
    F32R = mybir.dt.float32r
    BF16 = mybir.dt.bfloat16
    ALU = mybir.AluOpType
    AF = mybir.ActivationFunctionType

    nc = bacc.Bacc("TRN2", debug=False, num_devices=1 if single else NCORES)

    xq = nc.dram_tensor("xq", [R, D], F32, kind="ExternalInput").ap()
    xk = nc.dram_tensor("xk", [R, D], F32, kind="ExternalInput").ap()
    xv = nc.dram_tensor("xv", [R, D], F32, kind="ExternalInput").ap()
    wq = nc.dram_tensor("wq", [D, D], F32, kind="ExternalInput").ap()
    wk = nc.dram_tensor("wk", [D, D], F32, kind="ExternalInput").ap()
    wv = nc.dram_tensor("wv", [D, D], F32, kind="ExternalInput").ap()
    wo = nc.dram_tensor("wo", [D, D], F32, kind="ExternalInput").ap()
    bq = nc.dram_tensor("bq", [1, D], F32, kind="ExternalInput").ap()
    bk = nc.dram_tensor("bk", [1, D], F32, kind="ExternalInput").ap()
    bv = nc.dram_tensor("bv", [1, D], F32, kind="ExternalInput").ap()
    bo = nc.dram_tensor("bo", [1, D], F32, kind="ExternalInput").ap()
    gamma = nc.dram_tensor("gamma", [1, D], F32, kind="ExternalInput").ap()
    beta = nc.dram_tensor("beta", [1, D], F32, kind="ExternalInput").ap()
    out = nc.dram_tensor("out", [R, D], F32, kind="ExternalOutput").ap()

    with tile.TileContext(nc) as tc, ExitStack() as ctx:
        const_p = ctx.enter_context(tc.tile_pool(name="const", bufs=1))
        stage = ctx.enter_context(tc.tile_pool(name="stage", bufs=3))
        dp = ctx.enter_context(tc.tile_pool(name="dram", bufs=1, space="DRAM"))

        # ---- constants ----
        ident = const_p.tile([128, 128], F32, tag="ident")
        make_identity(nc, ident[:])

        ones_f = const_p.tile([128, 1], F32, tag="ones_f")
        nc.gpsimd.memset(ones_f[:], 1.0)
        ones_r = const_p.tile([128, 1], F32R, tag="ones_r")
        nc.scalar.copy(ones_r[:], ones_f[:])

        eps_ln = const_p.tile([128, 1], F32, tag="eps_ln")
        nc.gpsimd.memset(eps_ln[:], EPS_LN)

        # S selection matrix: S[h, x] = 1 iff h == x // 64
        s_f = stage.tile([16, D], F32, tag="wstage", padded_shape=[128, D])
        nc.gpsimd.memset(s_f[:], 0.0)
        s_f3 = s_f[:].rearrange("h (j l) -> h j l", l=64)
        nc.gpsimd.affine_select(
            out=s_f3,
            in_=s_f3,
            compare_op=ALU.not_equal,
            fill=1.0,
            base=0,
            pattern=[[-1, 16], [0, 64]],
            channel_multiplier=1,
        )
        s_r = const_p.tile([16, D], F32R, tag="s_r")
        nc.scalar.copy(s_r[:], s_f[:])

        def bcast_row(name, src_ap):
            row = const_p.tile([1, D], F32, tag=name + "_row")
            nc.sync.dma_start(row[:], src_ap)
            bc = const_p.tile([128, D], F32, tag=name + "_bc")
            nc.gpsimd.partition_broadcast(bc[:], row[:])
            return bc

        bq_bc = None if zb_qk else bcast_row("bq", bq)
        bk_bc = None if zb_qk else bcast_row("bk", bk)
        bo_bc = None if zb_o else bcast_row("bo", bo)
        gamma_bc = None if g_one else bcast_row("gamma", gamma)
        beta_bc = None if b_zero else bcast_row("beta", beta)
        bv_pp = None
        if not zb_v:
            # per-partition bias for feature-major V: bv_pp[p, c] = bv[c*128+p]
            bv_pp = const_p.tile([128, 8], F32, tag="bv_pp")
            for c in range(8):
                nc.sync.dma_start(
                    bv_pp[:, c : c + 1], bv[0:1, c * 128 : (c + 1) * 128]
                )

        def load_weights(pool, w_ap, name):
            tiles = []
            for c in range(8):
                st = stage.tile([128, D], F32, tag="wstage")
                nc.gpsimd.dma_start(st[:], w_ap[c * 128 : (c + 1) * 128, :])
                wt = pool.tile([128, D], F32R, tag=f"{name}{c}")
                nc.scalar.copy(wt[:], st[:])
                tiles.append(wt)
            return tiles

        # K scratch in HBM (token-major, f32r-rounded values stored as f32 bits)
        kscr = dp.tile([R, D], BF16, tag="kscr")
        cc_in = dp.tile([64, 1040], F32, tag="cc_in")
        cc_out = dp.tile([64, 1040], F32, tag="cc_out")

        wB = ctx.enter_context(tc.tile_pool(name="wB", bufs=1))
        # =========================== PHASE A ===========================
        from contextlib import ExitStack as _ES
        with _ES() as actx:
            wA = actx.enter_context(tc.tile_pool(name="wA", bufs=1))
            xanat = actx.enter_context(tc.tile_pool(name="xanat", bufs=2))
            xaT = actx.enter_context(tc.tile_pool(name="xaT", bufs=6))
            qksb = actx.enter_context(tc.tile_pool(name="qksb", bufs=3))
            elu_t = actx.enter_context(tc.tile_pool(name="elu_t", bufs=3))
            rsb = actx.enter_context(tc.tile_pool(name="rsb", bufs=1))
            psA = actx.enter_context(tc.tile_pool(name="psA", bufs=3, space="PSUM"))
            psTr = actx.enter_context(tc.tile_pool(name="psTr", bufs=4, space="PSUM"))
            psKtq = actx.enter_context(tc.tile_pool(name="psKtq", bufs=1, space="PSUM"))
            wq_r = load_weights(wA, wq, "wq")
            wk_r = load_weights(wA, wk, "wk")
            wv_r = load_weights(wB, wv, "wv")

            ktq_acc = rsb.tile([64, 1024], F32, tag="ktq_acc")
            nc.vector.memset(ktq_acc[:], 0.0)
            q_acc = rsb.tile([128, 1024], F32, tag="q_acc")
            nc.gpsimd.memset(q_acc[:], 0.0)

            def project(x_nat, w_tiles, ps_tag):
                # 4 PE transposes per psum bank, drained by one DVE copy
                xt_groups = []
                for g in range(2):
                    ps_t = psTr.tile([128, 512], F32, tag="trA")
                    for j in range(4):
                        c = g * 4 + j
                        nc.tensor.transpose(
                            ps_t[:, j * 128 : (j + 1) * 128],
                            x_nat[:, c * 128 : (c + 1) * 128],
                            ident[:],
                        )
                    xt = xaT.tile([128, 512], F32R, tag="xaT")
                    nc.scalar.copy(xt[:], ps_t[:])
                    xt_groups.append(xt)
                xt_tiles = [
                    xt_groups[c // 4][:, (c % 4) * 128 : (c % 4 + 1) * 128]
                    for c in range(8)
                ]
                halves = []
                for of in range(2):
                    ph = psA.tile([128, 512], F32, tag="ps_qk", name=f"{ps_tag}_{of}")
                    for c in range(8):
                        nc.tensor.matmul(
                            ph[:],
                            xt_tiles[c],
                            w_tiles[c][:, of * 512 : (of + 1) * 512],
                            start=(c == 0),
                            stop=(c == 7),
                        )
                    halves.append(ph)
                return halves

            def elu_p1(halves, dst, bias_bc):
                """dst[128,1024](f32r) = elu(ps + bias) + 1
                = max(x + 1, min(exp(x), 1))."""
                for of in range(2):
                    sl = slice(of * 512, (of + 1) * 512)
                    src = halves[of][:]
                    if bias_bc is not None:
                        xb = elu_t.tile([128, 512], F32, tag="xb")
                        nc.vector.tensor_tensor(xb[:], src, bias_bc[:, sl], ALU.add)
                        src = xb[:]
                    e = elu_t.tile([128, 512], F32, tag="e")
                    nc.scalar.activation(e[:], src, AF.Exp)
                    em1 = elu_t.tile([128, 512], F32, tag="em1")
                    nc.vector.tensor_scalar_min(em1[:], e[:], 1.0)
                    nc.vector.scalar_tensor_tensor(
                        dst[:, sl], src, 1.0, em1[:], op0=ALU.add, op1=ALU.max
                    )

            def reduce_subtile(s, q_sb, k_sb):
                """KtQ per-subtile psum (one bank at a time), accumulated
                into SBUF by DVE. (interleaved long-lived psum groups are
                unsafe: start=True clears has_written for the WHOLE bank)"""
                rows = slice(s * 128, (s + 1) * 128)
                for half in range(2):
                    ktq_ps = psKtq.tile([64, 512], F32, tag="ktq_ps")
                    for hh in range(8):
                        h = half * 8 + hh
                        nc.tensor.matmul(
                            ktq_ps[:, hh * 64 : (hh + 1) * 64],
                            k_sb[:, h * 64 : (h + 1) * 64],
                            q_sb[:, h * 64 : (h + 1) * 64],
                            start=True,
                            stop=True,
                        )
                    nc.vector.tensor_tensor(
                        ktq_acc[:, half * 512 : (half + 1) * 512],
                        ktq_acc[:, half * 512 : (half + 1) * 512],
                        ktq_ps[:],
                        ALU.add,
                    )
                nc.gpsimd.tensor_tensor(q_acc[:], q_acc[:], q_sb[:], ALU.add)
                nc.sync.dma_start(kscr[rows, :], k_sb[:])

            pending = None  # software pipeline: reductions trail by 1 subtile
            for s in range(NSUB):
                rows = slice(s * 128, (s + 1) * 128)
                xq_nat = xanat.tile([128, 1024], F32, tag="xq_nat")
                nc.sync.dma_start(xq_nat[:], xq[rows, :])
                xk_nat = xanat.tile([128, 1024], F32, tag="xk_nat")
                nc.sync.dma_start(xk_nat[:], xk[rows, :])

                ps_q = project(xq_nat, wq_r, "ps_q")
                q_sb = qksb.tile([128, 1024], BF16, tag="q_sb")
                elu_p1(ps_q, q_sb, bq_bc)

                ps_k = project(xk_nat, wk_r, "ps_k")
                k_sb = qksb.tile([128, 1024], BF16, tag="k_sb")
                elu_p1(ps_k, k_sb, bk_bc)

                if pending is not None:
                    reduce_subtile(*pending)
                pending = (s, q_sb, k_sb)
            reduce_subtile(*pending)

            qsum_sb = rsb.tile([1, 1024], F32, tag="qsum_sb")
            for of in range(2):
                qps = psTr.tile(
                    [1, 512], F32, tag="trA", name=f"qsum_ps_{of}",
                    padded_shape=[128, 512],
                )
                nc.tensor.matmul(
                    qps[:],
                    ones_f[:],
                    q_acc[:, of * 512 : (of + 1) * 512],
                    start=True,
                    stop=True,
                )
                nc.scalar.copy(qsum_sb[0:1, of * 512 : (of + 1) * 512], qps[:])

            nc.sync.dma_start(cc_in[0:64, 0:1024], ktq_acc[:])
            # q_sum[f] -> cc row f//16, col 1024 + f%16
            nc.sync.dma_start(cc_in[0:64, 1024:1040], qsum_sb[:])

        if single:
            nc.sync.dma_start(cc_out[:], cc_in[:])
        else:
            nc.gpsimd.collective_compute(
                "AllReduce",
                ALU.add,
                replica_groups=[[0, 1], [2, 3], [4, 5], [6, 7]],
                ins=[cc_in.opt()],
                outs=[cc_out.opt()],
            )

        # =========================== PHASE B ===========================
        with _ES() as bctx:
            wBo = bctx.enter_context(tc.tile_pool(name="wBo", bufs=1))
            redu = bctx.enter_context(tc.tile_pool(name="redu", bufs=1))
            xbnat = bctx.enter_context(tc.tile_pool(name="xbnat", bufs=3))
            xvT_p = bctx.enter_context(tc.tile_pool(name="xvT", bufs=8))
            xvnat_p = bctx.enter_context(tc.tile_pool(name="xvnat", bufs=4))
            knat_p = bctx.enter_context(tc.tile_pool(name="knat", bufs=3))
            vsb_p = bctx.enter_context(tc.tile_pool(name="vsb", bufs=8))
            zt_p = bctx.enter_context(tc.tile_pool(name="zt", bufs=2))
            div_p = bctx.enter_context(tc.tile_pool(name="divsb", bufs=1))
            oh_p = bctx.enter_context(tc.tile_pool(name="ohsb", bufs=8))
            lnt = bctx.enter_context(tc.tile_pool(name="lnt", bufs=2))
            small = bctx.enter_context(tc.tile_pool(name="small", bufs=4))
            ostage = bctx.enter_context(tc.tile_pool(name="ostage", bufs=2))
            psV = bctx.enter_context(tc.tile_pool(name="psV", bufs=2, space="PSUM"))
            psNum = bctx.enter_context(tc.tile_pool(name="psNum", bufs=2, space="PSUM"))
            psDiv = bctx.enter_context(tc.tile_pool(name="psDiv", bufs=1, space="PSUM"))
            psAttn = bctx.enter_context(tc.tile_pool(name="psAttn", bufs=2, space="PSUM"))
            psTrB = bctx.enter_context(tc.tile_pool(name="psTrB", bufs=1, space="PSUM"))
            def phase_b_setup():
                # block-diagonal KtQ: pair c -> [128,128] block, off-diag zero
                ktq_f = stage.tile([128, 1024], F32, tag="wstage", name="ktq_f")
                nc.gpsimd.memset(ktq_f[:], 0.0)
                bd3 = ktq_f[:].rearrange("p (c e) -> p c e", e=128)
                cc3 = cc_out[0:64, 0:1024].rearrange(
                    "p (c t e) -> p c t e", t=2, e=64
                )
                nc.sync.dma_start(bd3[0:64, :, 0:64], cc3[:, :, 0, :])
                nc.sync.dma_start(bd3[64:128, :, 64:128], cc3[:, :, 1, :])
                ktq_r = redu.tile([128, 1024], F32R, tag="ktq_r")
                nc.scalar.copy(ktq_r[:], ktq_f[:])

                qsum_lin = stage.tile(
                    [1, 1024], F32, tag="wstage", name="qsum_lin",
                    padded_shape=[128, 1024],
                )
                nc.sync.dma_start(qsum_lin[:], cc_out[0:64, 1024:1040])
                qsum_bc = redu.tile([128, 1024], F32, tag="qsum_bc")
                nc.gpsimd.partition_broadcast(qsum_bc[:], qsum_lin[:])
                return ktq_r, qsum_bc

            ktq_r = qsum_bc = wo_r = None
            for blk in range(NBLK):
                # ---- V projection (feature-major) ----
                xv_nats = []
                for t in range(4):
                    rows = slice(blk * 512 + t * 128, blk * 512 + (t + 1) * 128)
                    xv_nat = xvnat_p.tile(
                        [128, 1024], F32, tag="xv_nat", name=f"xv_nat_{blk}_{t}"
                    )
                    nc.sync.dma_start(xv_nat[:], xv[rows, :])
                    xv_nats.append(xv_nat)
                xv_t = []
                for c in range(8):
                    ps_t = psTrB.tile([128, 512], F32, tag="trB")
                    for t in range(4):
                        nc.tensor.transpose(
                            ps_t[:, t * 128 : (t + 1) * 128],
                            xv_nats[t][:, c * 128 : (c + 1) * 128],
                            ident[:],
                        )
                    xt = xvT_p.tile([128, 512], F32R, tag="xvT", name=f"xvt_{blk}_{c}")
                    nc.vector.tensor_copy(xt[:], ps_t[:])
                    xv_t.append(xt)

                if blk == 0:
                    wo_r = load_weights(wBo, wo, "wo")
                    ktq_r, qsum_bc = phase_b_setup()

                # ---- Z = K . q_sum (per token, per head) via K reload ----
                invz_fm = small.tile([16, 512], F32R, tag="invz_fm")
                for t in range(4):
                    rows = slice(blk * 512 + t * 128, blk * 512 + (t + 1) * 128)
                    k_nat = knat_p.tile(
                        [128, 1024], BF16, tag="k_nat", name=f"k_nat_{blk}_{t}"
                    )
                    nc.sync.dma_start(k_nat[:], kscr[rows, :])
                    prod = zt_p.tile([128, 1024], F32, tag="prod")
                    nc.vector.tensor_tensor(prod[:], k_nat[:], qsum_bc[:], ALU.mult)
                    z_t = zt_p.tile([128, 16], F32, tag="z_t")
                    nc.vector.tensor_reduce(
                        z_t[:],
                        prod[:].rearrange("p (h e) -> p h e", e=64),
                        mybir.AxisListType.X,
                        ALU.add,
                    )
                    z_e = zt_p.tile([128, 16], F32, tag="z_e")
                    nc.vector.tensor_scalar_add(z_e[:], z_t[:], EPS_Z)
                    iz_t = zt_p.tile([128, 16], F32, tag="iz_t")
                    nc.vector.reciprocal(iz_t[:], z_e[:])
                    ps_zt = psTrB.tile([128, 512], F32, tag="trB")
                    nc.tensor.transpose(ps_zt[0:16, 0:128], iz_t[:], ident[:])
                    nc.scalar.copy(
                        invz_fm[:, t * 128 : (t + 1) * 128], ps_zt[0:16, 0:128]
                    )

                v_sb = []
                for c in range(8):
                    ps_v = psV.tile([128, 512], F32, tag="ps_v")
                    for kk in range(8):
                        nc.tensor.matmul(
                            ps_v[:],
                            wv_r[kk][:, c * 128 : (c + 1) * 128],
                            xv_t[kk][:],
                            start=(kk == 0),
                            stop=(kk == 7),
                        )
                    vt = vsb_p.tile([128, 512], F32R, tag="v_sb")
                    if zb_v:
                        nc.scalar.copy(vt[:], ps_v[:])
                    else:
                        nc.scalar.activation(
                            vt[:], ps_v[:], AF.Identity, bias=bv_pp[:, c : c + 1]
                        )
                    v_sb.append(vt)

                # ---- numerator + divisor broadcast + OH = num/Z ----
                oh_sb = []
                for c in range(8):
                    ps_n = psNum.tile([128, 512], F32, tag="ps_n")
                    nc.tensor.matmul(
                        ps_n[:],
                        ktq_r[:, c * 128 : (c + 1) * 128],
                        v_sb[c][:],
                        start=True,
                        stop=True,
                    )
                    ps_d = psDiv.tile([128, 512], F32, tag="ps_d")
                    nc.tensor.matmul(
                        ps_d[:],
                        s_r[:, c * 128 : (c + 1) * 128],
                        invz_fm[:],
                        start=True,
                        stop=True,
                    )
                    div_sb = div_p.tile([128, 512], F32, tag="div_sb")
                    nc.scalar.copy(div_sb[:], ps_d[:])
                    oh = oh_p.tile([128, 512], F32R, tag="oh")
                    nc.vector.scalar_tensor_tensor(
                        oh[:], ps_n[:], 1.0, div_sb[:], op0=ALU.mult, op1=ALU.mult
                    )
                    oh_sb.append(oh)

                # ---- out-projection + residual + LayerNorm ----
                for t in range(4):
                    rows = slice(blk * 512 + t * 128, blk * 512 + (t + 1) * 128)
                    q_nat = xbnat.tile([128, 1024], F32, tag="xnat", name=f"q_nat_{blk}_{t}")
                    nc.sync.dma_start(q_nat[:], xq[rows, :])
                    res = q_nat[:]
                    if bo_bc is not None:
                        qb = lnt.tile([128, 1024], F32, tag="qb")
                        nc.vector.tensor_tensor(qb[:], q_nat[:], bo_bc[:], ALU.add)
                        res = qb[:]

                    x_sb = lnt.tile([128, 1024], F32, tag="x_sb")
                    s1 = small.tile([128, 2], F32, tag="s1")
                    for of in range(2):
                        sl = slice(of * 512, (of + 1) * 512)
                        ps_a = psAttn.tile(
                            [128, 512], F32, tag="ps_a", name=f"ps_a_{blk}_{t}_{of}"
                        )
                        for c in range(8):
                            nc.tensor.matmul(
                                ps_a[:],
                                oh_sb[c][:, t * 128 : (t + 1) * 128],
                                wo_r[c][:, of * 512 : (of + 1) * 512],
                                start=(c == 0),
                                stop=(c == 7),
                            )
                        nc.vector.scalar_tensor_tensor(
                            x_sb[:, sl], ps_a[:], 1.0, res[:, sl],
                            op0=ALU.mult, op1=ALU.add,
                            accum_out=s1[:, of : of + 1],
                        )
                    s1t = small.tile([128, 1], F32, tag="s1t")
                    nc.vector.tensor_reduce(
                        s1t[:], s1[:], mybir.AxisListType.X, ALU.add
                    )
                    mu = small.tile([128, 1], F32, tag="mu")
                    nc.scalar.mul(mu[:], s1t[:], 1.0 / D)
                    y = ostage.tile([128, 1024], F32, tag="y")
                    s2 = small.tile([128, 1], F32, tag="s2")
                    # y is scratch here; overwritten below
                    nc.scalar.activation(y[:], x_sb[:], AF.Square, accum_out=s2[:])
                    mu2 = small.tile([128, 1], F32, tag="mu2")
                    nc.scalar.square(mu2[:], mu[:])
                    var = small.tile([128, 1], F32, tag="var")
                    nc.vector.tensor_scalar(
                        var[:], s2[:], 1.0 / D, mu2[:], op0=ALU.mult, op1=ALU.subtract
                    )
                    std = small.tile([128, 1], F32, tag="std")
                    nc.scalar.activation(std[:], var[:], AF.Sqrt, bias=eps_ln[:])
                    rstd = small.tile([128, 1], F32, tag="rstd")
                    nc.vector.reciprocal(rstd[:], std[:])

                    nc.vector.tensor_scalar(
                        y[:], x_sb[:], mu[:], rstd[:],
                        op0=ALU.subtract, op1=ALU.mult,
                    )
                    if not g_one:
                        nc.vector.tensor_tensor(y[:], y[:], gamma_bc[:], ALU.mult)
                    if not b_zero:
                        nc.vector.tensor_tensor(y[:], y[:], beta_bc[:], ALU.add)
                    nc.sync.dma_start(out[rows, :], y[:])

    nc.compile()
    return nc


def _get_nc(flags):
    if flags not in _CACHE:
        _CACHE[flags] = _build(*flags)
    return _CACHE[flags]


def _prep(inputs):
    q = np.ascontiguousarray(np.asarray(inputs["query"], dtype=np.float32))
    k = np.ascontiguousarray(np.asarray(inputs["key"], dtype=np.float32))
    v = np.ascontiguousarray(np.asarray(inputs["value"], dtype=np.float32))
    Wq = np.ascontiguousarray(np.asarray(inputs["Wq"], dtype=np.float32))
    Wk = np.ascontiguousarray(np.asarray(inputs["Wk"], dtype=np.float32))
    Wv = np.ascontiguousarray(np.asarray(inputs["Wv"], dtype=np.float32))
    Wo = np.ascontiguousarray(np.asarray(inputs["Wo"], dtype=np.float32))
    bqv = np.ascontiguousarray(np.asarray(inputs["bq"], dtype=np.float32).reshape(1, D))
    bkv = np.ascontiguousarray(np.asarray(inputs["bk"], dtype=np.float32).reshape(1, D))
    bvv = np.ascontiguousarray(np.asarray(inputs["bv"], dtype=np.float32).reshape(1, D))
    bov = np.ascontiguousarray(np.asarray(inputs["bo"], dtype=np.float32).reshape(1, D))
    gv = np.ascontiguousarray(np.asarray(inputs["gamma"], dtype=np.float32).reshape(1, D))
    btv = np.ascontiguousarray(np.asarray(inputs["beta"], dtype=np.float32).reshape(1, D))

    flags = (
        bool(not bqv.any() and not bkv.any()),
        bool(not bvv.any()),
        bool(not bov.any()),
        bool(np.all(gv == 1.0)),
        bool(not btv.any()),
    )
    qf = q.reshape(NCORES, R, D)
    kf = k.reshape(NCORES, R, D)
    vf = v.reshape(NCORES, R, D)
    in_maps = []
    for c in range(NCORES):
        in_maps.append(
            {
                "xq": qf[c], "xk": kf[c], "xv": vf[c],
                "wq": Wq, "wk": Wk, "wv": Wv, "wo": Wo,
                "bq": bqv, "bk": bkv, "bv": bvv, "bo": bov,
                "gamma": gv, "beta": btv,
            }
        )
    return flags, in_maps


def kernel(**inputs):
    from concourse.bass_utils import run_bass_kernel_spmd

    flags, in_maps = _prep(inputs)
    nc = _get_nc(flags)
    res = run_bass_kernel_spmd(nc, in_maps, core_ids=list(range(NCORES)))
    outs = np.stack([res.results[c]["out"] for c in range(NCORES)], axis=0)
    return outs.reshape(B, N, D)



# revision 70
# speedup vs baseline: 1.6440x; 1.6440x over previous
"""Trainium2 Bass kernel for linear attention (ELU+1 feature map) block:
Q/K/V projections + linear attention + out-projection + residual + LayerNorm,
distributed over 8 NeuronCores.

Sharding: 8-way row split of (batch*seq); cores 2b, 2b+1 hold the two
2048-token halves of batch b. Per-(batch,head) global reductions (ones-
augmented K^T.Q) are pair-AllReduced. Weights are row-sliced across all 8
cores on the host, cast to fp8 on device, and AllGathered on-chip.

Projections run as fp8e4 DoubleRow matmuls (2 contraction planes per
instruction, 0.5 cyc/row). The feature map is computed as psi = elu(x)
(not elu+1); the ones column of the augmented K matmul reconstructs all
the (psi+1) cross terms after the collective.
"""
import os
import sys

for _p in ("/opt/trn_rl_repo", "/root/.axon_site/_ro/trn_rl_repo"):
    if os.path.isdir(_p) and _p not in sys.path:
        sys.path.insert(0, _p)

import numpy as np

B, N, D, H = 4, 4096, 1024, 16
DEPTH = D // H  # 64
NCORES = 8
R = (B * N) // NCORES  # 2048 rows per core
NSUB = R // 128  # 16 token subtiles per core
NBLK = R // 512  # 4 token blocks in phase B
EPS_Z = 1e-9
EPS_LN = 1e-6
WS = 64.0        # weight pre-scale before fp8 cast
OHS = 16.0       # oh pre-scale before fp8 cast

_CACHE = {}


def _build(zb_qk, zb_v, zb_o, g_one, b_zero, single=False):
    import concourse.bacc as bacc
    import concourse.tile as tile
    from concourse import mybir
    from concourse.masks import make_identity
    from contextlib import ExitStack

    F32 = mybir.dt.float32
    BF16 = mybir.dt.bfloat16
    FP8 = mybir.dt.float8e4
    ALU = mybir.AluOpType
    AF = mybir.ActivationFunctionType
    DR = mybir.MatmulPerfMode.DoubleRow

    nc = bacc.Bacc("TRN2", debug=False, num_devices=1 if single else NCORES)

    xq = nc.dram_tensor("xq", [R, D], F32, kind="ExternalInput").ap()
    xkt = nc.dram_tensor("xkt", [D, R], F32, kind="ExternalInput").ap()
    xvt = nc.dram_tensor("xvt", [D, R], F32, kind="ExternalInput").ap()
    # host-sliced weight rows [c*128:(c+1)*128] of each W
    wq_s = nc.dram_tensor("wq_s", [128, D], F32, kind="ExternalInput").ap()
    wk_s = nc.dram_tensor("wk_s", [128, D], F32, kind="ExternalInput").ap()
    wv_s = nc.dram_tensor("wv_s", [128, D], F32, kind="ExternalInput").ap()
    wo_s = nc.dram_tensor("wo_s", [128, D], F32, kind="ExternalInput").ap()
    bq = nc.dram_tensor("bq", [1, D], F32, kind="ExternalInput").ap()
    bk = nc.dram_tensor("bk", [1, D], F32, kind="ExternalInput").ap()
    bv = nc.dram_tensor("bv", [1, D], F32, kind="ExternalInput").ap()
    bo = nc.dram_tensor("bo", [1, D], F32, kind="ExternalInput").ap()
    gamma = nc.dram_tensor("gamma", [1, D], F32, kind="ExternalInput").ap()
    beta = nc.dram_tensor("beta", [1, D], F32, kind="ExternalInput").ap()
    out = nc.dram_tensor("out", [R, D], BF16, kind="ExternalOutput").ap()

    with tile.TileContext(nc) as tc, ExitStack() as ctx:
        const_p = ctx.enter_context(tc.tile_pool(name="const", bufs=1))
        dp = ctx.enter_context(tc.tile_pool(name="dram", bufs=1, space="DRAM"))
        # long-lived SBUF state
        xq_pool = ctx.enter_context(tc.tile_pool(name="xqn", bufs=1))
        ksb_pool = ctx.enter_context(tc.tile_pool(name="ksb", bufs=1))
        red_pool = ctx.enter_context(tc.tile_pool(name="red", bufs=1))
        wB = ctx.enter_context(tc.tile_pool(name="wB", bufs=1))
        xvt_f_p = ctx.enter_context(tc.tile_pool(name="xvtf", bufs=2))
        xvt_q_p = ctx.enter_context(tc.tile_pool(name="xvtq", bufs=2))

        # ---- weight slice cast + AllGather (first: gates phase A) ----
        cc_w_in = dp.tile([128, 3 * D], FP8, tag="cc_w_in")
        cc_w_out = dp.tile([8 * 128, 3 * D], FP8, tag="cc_w_out")
        cc_wv_in = dp.tile([128, D], BF16, tag="cc_wv_in")
        cc_wv_out = dp.tile([8 * 128, D], BF16, tag="cc_wv_out")
        cc_kq_in = dp.tile([65, D], BF16, tag="cc_kq_in")
        cc_kq_out = dp.tile([65, D], BF16, tag="cc_kq_out")

        s_sel = const_p.tile([16, D], BF16, tag="s_sel")

        from contextlib import ExitStack as _ES0
        with _ES0() as initctx:
            stage = initctx.enter_context(tc.tile_pool(name="stage", bufs=2))
            wslice_q = stage.tile([128, 3 * D], FP8, tag="wslice_q", name="wslice_q")
            for i, w_ap in enumerate((wq_s, wk_s, wo_s)):
                st = stage.tile([128, D], F32, tag="wstage", name=f"wsl_{i}")
                nc.sync.dma_start(st[:], w_ap)
                nc.vector.tensor_scalar_mul(
                    wslice_q[:, i * D : (i + 1) * D], st[:], WS
                )
            nc.sync.dma_start(cc_w_in[:], wslice_q[:])
            # wv slice in bf16 (V path needs the precision; see precsim)
            wv_st = stage.tile([128, D], F32, tag="wstage", name="wsl_v")
            nc.sync.dma_start(wv_st[:], wv_s)
            wv_bf = stage.tile([128, D], BF16, tag="wv_bf", name="wv_bf")
            nc.vector.tensor_copy(wv_bf[:], wv_st[:])
            nc.sync.dma_start(cc_wv_in[:], wv_bf[:])

            if single:
                nc.sync.dma_start(cc_w_out[0:128, :], cc_w_in[:])
                nc.sync.dma_start(cc_wv_out[0:128, :], cc_wv_in[:])
            else:
                nc.gpsimd.collective_compute(
                    "AllGather", ALU.bypass,
                    replica_groups=[list(range(NCORES))],
                    ins=[cc_w_in.opt()], outs=[cc_w_out.opt()],
                )
                nc.gpsimd.collective_compute(
                    "AllGather", ALU.bypass,
                    replica_groups=[list(range(NCORES))],
                    ins=[cc_wv_in.opt()], outs=[cc_wv_out.opt()],
                )

            # S selection matrix (bf16): s_sel[h, f] = 1 iff h == head(f)
            s_f = stage.tile([16, D], F32, tag="wstage", name="s_build",
                             padded_shape=[128, D])
            nc.gpsimd.memset(s_f[:], 0.0)
            s_f3 = s_f[:].rearrange("h (j l) -> h j l", l=64)
            nc.gpsimd.affine_select(
                out=s_f3, in_=s_f3, compare_op=ALU.not_equal, fill=1.0,
                base=0, pattern=[[-1, 16], [0, 64]], channel_multiplier=1,
            )
            nc.vector.tensor_copy(s_sel[:], s_f[:])

        # ---- constants ----
        ident = const_p.tile([128, 128], F32, tag="ident")
        make_identity(nc, ident[:])
        ones_bf = const_p.tile([128, 1], BF16, tag="ones_bf")
        nc.gpsimd.memset(ones_bf[:], 1.0)
        eps_ln = const_p.tile([128, 1], F32, tag="eps_ln")
        nc.gpsimd.memset(eps_ln[:], EPS_LN)

        def bcast_row(name, src_ap):
            row = const_p.tile([1, D], F32, tag=name + "_row")
            nc.sync.dma_start(row[:], src_ap)
            bc = const_p.tile([128, D], F32, tag=name + "_bc")
            nc.gpsimd.partition_broadcast(bc[:], row[:])
            return bc

        bq_bc = None if zb_qk else bcast_row("bq", bq)
        bk_bc = None if zb_qk else bcast_row("bk", bk)
        bo_bc = None if zb_o else bcast_row("bo", bo)
        gamma_bc = None if g_one else bcast_row("gamma", gamma)
        beta_bc = None if b_zero else bcast_row("beta", beta)
        bv_pp = None
        if not zb_v:
            bv_pp = const_p.tile([128, 8], F32, tag="bv_pp")
            for c in range(8):
                nc.sync.dma_start(bv_pp[:, c : c + 1], bv[0:1, c * 128 : (c + 1) * 128])

        # gathered weights -> [128, 8, D] fp8 tiles (k-plane-major pairs)
        cc_w3 = cc_w_out[:].rearrange("(cb p) n -> p cb n", p=128)

        def load_w(pool, widx, name):
            # ACT-queue DMA: waits on the gather without blocking the x
            # prefetch stream on the sync queue
            wt = pool.tile([128, 8, D], FP8, tag=name, name=name)
            nc.scalar.dma_start(wt[:], cc_w3[:, :, widx * D : (widx + 1) * D])
            return wt

        xvt3 = xvt.rearrange("(cb p) m -> p cb m", p=128)

        def load_xvt(blk):
            xvt_q = xvt_q_p.tile([128, 8, 512], BF16, tag="xvt_q",
                                 name=f"xvt_q{blk}")
            for hb in range(4):
                cols = slice(blk * 512 + hb * 128, blk * 512 + (hb + 1) * 128)
                xvt_f = xvt_f_p.tile([128, 8, 128], F32, tag="xvt_f",
                                     name=f"xvt_f{blk}_{hb}")
                nc.sync.dma_start(xvt_f[:], xvt3[:, :, cols])
                nc.gpsimd.tensor_copy(
                    xvt_q[:, :, hb * 128 : (hb + 1) * 128], xvt_f[:]
                )
            return xvt_q

        # ---- persistent activation state ----
        # k_sb[s]: [128 tok, 16 heads, 65] bf16; col 64 = 1.0 (ones augment)
        k_tiles = []
        for s in range(NSUB):
            kt = ksb_pool.tile([128, 16, 65], BF16, tag=f"k_sb{s}", name=f"k_sb{s}")
            nc.gpsimd.memset(kt[:, :, 64:65], 1.0)
            k_tiles.append(kt)
        xq_tiles = [
            xq_pool.tile([128, D], F32, tag=f"xq_nat{s}", name=f"xq_nat{s}")
            for s in range(NSUB)
        ]

        # KtQ rows 0:64 = phiK^T phiQ, row 64 = colsum phiQ (= q_sum)
        ktq_acc = red_pool.tile([65, D], BF16, tag="ktq_acc", padded_shape=[128, D])

        # =========================== PHASE A ===========================
        from contextlib import ExitStack as _ES
        with _ES() as actx:
            wA = actx.enter_context(tc.tile_pool(name="wA", bufs=1))
            wq_t = load_w(wA, 0, "wq_t")
            wk_t = load_w(wA, 1, "wk_t")
            xkt_f_p = actx.enter_context(tc.tile_pool(name="xktf", bufs=2))
            xt_q_p = actx.enter_context(tc.tile_pool(name="xtq", bufs=3))
            qsb_p = actx.enter_context(tc.tile_pool(name="qsb", bufs=3))
            elu_p = actx.enter_context(tc.tile_pool(name="elu", bufs=4))
            psTr = actx.enter_context(tc.tile_pool(name="psTr", bufs=2, space="PSUM"))
            psQK = actx.enter_context(tc.tile_pool(name="psQK", bufs=4, space="PSUM"))
            psKtq = actx.enter_context(tc.tile_pool(name="psKtq", bufs=1, space="PSUM"))

            # KtQ accumulates across all 16 subtiles in 2 resident psum banks:
            # bank0 heads 0-7, bank1 heads 8-15 (cols h*64)
            HB = [(0, 8), (8, 16)]
            kp_banks = []
            for b, (h0, h1) in enumerate(HB):
                kpb = psKtq.tile(
                    [65, (h1 - h0) * 64], F32, tag=f"kp{b}", name=f"kp{b}",
                    padded_shape=[128, (h1 - h0) * 64],
                )
                kp_banks.append(kpb)

            xkt3 = xkt.rearrange("(cb p) m -> p cb m", p=128)

            def project(xt_q, w_t, ps_pool, tag):
                """fp8 DoubleRow projection: out [128 tok, 1024] psum halves."""
                halves = []
                for of in range(2):
                    ph = ps_pool.tile([128, 512], F32, tag="ps_qk", name=f"{tag}_{of}")
                    for u in range(4):
                        nc.tensor.matmul(
                            ph[:],
                            xt_q[:, 2 * u : 2 * u + 2, :],
                            w_t[:, 2 * u : 2 * u + 2, of * 512 : (of + 1) * 512],
                            start=(u == 0), stop=(u == 3),
                            perf_mode=DR,
                        )
                    halves.append(ph)
                return halves

            def phi(halves, dst3, bias_bc, relu_dve):
                """dst3[:, h, 0:64] (bf16) = elu(ps/WS)+1 = relu(ps/WS) + min(e, 1).
                relu on DVE (tensor_scalar) or ACT, to balance the engines."""
                for of in range(2):
                    src = halves[of][:]
                    if bias_bc is not None:
                        xb = elu_p.tile([128, 512], F32, tag="xb")
                        sl = slice(of * 512, (of + 1) * 512)
                        nc.vector.scalar_tensor_tensor(
                            xb[:], src, 1.0 / WS, bias_bc[:, sl],
                            op0=ALU.mult, op1=ALU.add,
                        )
                        src = xb[:]
                        scl = 1.0
                    else:
                        scl = 1.0 / WS
                    e = elu_p.tile([128, 512], BF16, tag="e")
                    nc.scalar.activation(e[:], src, AF.Exp, scale=scl)
                    r = elu_p.tile([128, 512], BF16, tag="r")
                    if relu_dve:
                        nc.vector.tensor_scalar(
                            r[:], src, scl, 0.0, op0=ALU.mult, op1=ALU.max
                        )
                    else:
                        nc.scalar.activation(r[:], src, AF.Relu, scale=scl)
                    dst = dst3[:, of * 8 : (of + 1) * 8, 0:64]
                    nc.vector.scalar_tensor_tensor(
                        dst, e[:], 1.0, r[:], op0=ALU.min, op1=ALU.add
                    )

            for s in range(NSUB):
                rows = slice(s * 128, (s + 1) * 128)
                xq_nat = xq_tiles[s]
                nc.sync.dma_start(xq_nat[:], xq[rows, :])
                xkt_f = xkt_f_p.tile([128, 8, 128], F32, tag="xkt_f")
                nc.sync.dma_start(xkt_f[:], xkt3[:, :, rows])
                xkt_q = xt_q_p.tile([128, 8, 128], FP8, tag="xkt_q")
                nc.gpsimd.tensor_copy(xkt_q[:], xkt_f[:])

                # transpose xq -> fp8 xqt (2 psum banks, 4 transposes each);
                # drains split across ACT/DVE
                xqt_q = xt_q_p.tile([128, 8, 128], FP8, tag="xqt_q")
                for g in range(2):
                    ps_t = psTr.tile([128, 512], F32, tag="trA")
                    for j in range(4):
                        c = g * 4 + j
                        nc.tensor.transpose(
                            ps_t[:, j * 128 : (j + 1) * 128],
                            xq_nat[:, c * 128 : (c + 1) * 128],
                            ident[:],
                        )
                    if g == 0:
                        nc.scalar.copy(xqt_q[:, 0:4, :], ps_t[:])
                    else:
                        nc.vector.tensor_copy(xqt_q[:, 4:8, :], ps_t[:])

                ps_k = project(xkt_q, wk_t, psQK, "ps_k")
                phi(ps_k, k_tiles[s][:], bk_bc, relu_dve=False)

                ps_q = project(xqt_q, wq_t, psQK, "ps_q")
                q_sb = qsb_p.tile([128, 16, 64], BF16, tag="q_sb")
                phi(ps_q, q_sb[:], bq_bc, relu_dve=True)

                # ones-augmented KtQ accumulated in resident psum banks:
                # out[d, (h, e)]: rows 0:64 phiK^T phiQ, row 64 colsum phiQ
                for b, (h0, h1) in enumerate(HB):
                    for h in range(h0, h1):
                        off = (h - h0) * 64
                        nc.tensor.matmul(
                            kp_banks[b][:, off : off + 64],
                            k_tiles[s][:, h, :],
                            q_sb[:, h, :],
                            start=(s == 0), stop=(s == NSUB - 1),
                        )

            # drain the resident KtQ psum banks
            for b, (h0, h1) in enumerate(HB):
                nc.scalar.copy(
                    ktq_acc[:, h0 * 64 : h1 * 64], kp_banks[b][:]
                )

            # prefetch V inputs for the first two phase-B blocks
            wv_t = wB.tile([128, 8, D], BF16, tag="wv_t", name="wv_t")
            nc.scalar.dma_start(
                wv_t[:], cc_wv_out[:].rearrange("(cb p) n -> p cb n", p=128)
            )
            wo_t = load_w(wB, 2, "wo_t")
            xvt_pre = {blk: load_xvt(blk) for blk in range(2)}

        nc.sync.dma_start(cc_kq_in[:], ktq_acc[:])
        if single:
            nc.sync.dma_start(cc_kq_out[:], cc_kq_in[:])
        else:
            nc.gpsimd.collective_compute(
                "AllReduce", ALU.add,
                replica_groups=[[0, 1], [2, 3], [4, 5], [6, 7]],
                ins=[cc_kq_in.opt()], outs=[cc_kq_out.opt()],
            )

        # =========================== PHASE B ===========================
        with _ES() as bctx:
            bred = bctx.enter_context(tc.tile_pool(name="bred", bufs=1))
            vsb_p = bctx.enter_context(tc.tile_pool(name="vsb", bufs=2))
            oh_p = bctx.enter_context(tc.tile_pool(name="oh", bufs=2))
            zt_p = bctx.enter_context(tc.tile_pool(name="zt", bufs=3))
            div_p = bctx.enter_context(tc.tile_pool(name="divsb", bufs=2))
            lnt = bctx.enter_context(tc.tile_pool(name="lnt", bufs=2))
            lns = bctx.enter_context(tc.tile_pool(name="lns", bufs=1))
            small = bctx.enter_context(tc.tile_pool(name="small", bufs=4))
            psV = bctx.enter_context(tc.tile_pool(name="psV", bufs=2, space="PSUM"))
            psNum = bctx.enter_context(tc.tile_pool(name="psNum", bufs=2, space="PSUM"))
            psDiv = bctx.enter_context(tc.tile_pool(name="psDiv", bufs=1, space="PSUM"))
            psO = bctx.enter_context(tc.tile_pool(name="psO", bufs=2, space="PSUM"))
            psIz = bctx.enter_context(tc.tile_pool(name="psIz", bufs=1, space="PSUM"))

            def phase_b_setup():
                """aug rows 0:64 are KtQ (phi cross-products), row 64 is q_sum.
                Build the block-diagonal bf16 lhsT and the q_sum broadcast."""
                qsum_bc = bred.tile([128, D], BF16, tag="qsum_bc")
                ktq_r = bred.tile([128, 8, 128], BF16, tag="ktq_r")
                with _ES() as sctx:
                    tmp = sctx.enter_context(tc.tile_pool(name="pbtmp", bufs=1))
                    rq_bf = tmp.tile([1, D], BF16, tag="rq_bf", padded_shape=[128, D])
                    nc.sync.dma_start(rq_bf[:], cc_kq_out[64:65, :])
                    nc.gpsimd.partition_broadcast(qsum_bc[:], rq_bf[:])

                    aug_bf = tmp.tile([64, D], BF16, tag="aug_bf",
                                      padded_shape=[128, D])
                    nc.sync.dma_start(aug_bf[:], cc_kq_out[0:64, :])
                    nc.gpsimd.memset(ktq_r[:], 0.0)
                    ktv = aug_bf[:].rearrange("d (c t e) -> d c t e", t=2, e=64)
                    nc.sync.dma_start(ktq_r[0:64, :, 0:64], ktv[:, :, 0, :])
                    nc.sync.dma_start(ktq_r[64:128, :, 64:128], ktv[:, :, 1, :])
                return ktq_r, qsum_bc

            ktq_r = qsum_bc = None

            for blk in range(NBLK):
                xvt_q = xvt_pre.pop(blk, None) or load_xvt(blk)
                if blk + 2 < NBLK:
                    xvt_pre[blk + 2] = load_xvt(blk + 2)

                if blk == 0:
                    ktq_r, qsum_bc = phase_b_setup()

                # ---- V projection (feature-major), bf16 for precision ----
                v_sb = vsb_p.tile([128, 8, 512], BF16, tag="v_sb")
                for c in range(8):
                    ps_v = psV.tile([128, 512], F32, tag="ps_v")
                    for u in range(8):
                        nc.tensor.matmul(
                            ps_v[:],
                            wv_t[:, u, c * 128 : (c + 1) * 128],
                            xvt_q[:, u, :],
                            start=(u == 0), stop=(u == 7),
                        )
                    if zb_v:
                        nc.scalar.copy(v_sb[:, c, :], ps_v[:])
                    else:
                        nc.scalar.activation(
                            v_sb[:, c, :], ps_v[:], AF.Identity,
                            bias=bv_pp[:, c : c + 1],
                        )

                # ---- z = psiK . qsum_phi + S (token-major), invz -> feat-major
                invz_fm = div_p.tile([16, 512], BF16, tag="invz_fm",
                                     padded_shape=[128, 512])
                for t in range(4):
                    s = blk * 4 + t
                    prod = zt_p.tile([128, 16, 64], BF16, tag="prod")
                    eng = nc.gpsimd if t % 2 == 0 else nc.vector
                    eng.tensor_tensor(
                        prod[:], k_tiles[s][:, :, 0:64],
                        qsum_bc[:].rearrange("p (h e) -> p h e", e=64),
                        ALU.mult,
                    )
                    zraw = zt_p.tile([128, 16], F32, tag="zraw")
                    nc.vector.tensor_reduce(
                        zraw[:], prod[:], mybir.AxisListType.X, ALU.add
                    )
                    z2 = zt_p.tile([128, 16], F32, tag="z2")
                    nc.vector.tensor_scalar_add(z2[:], zraw[:], EPS_Z)
                    iz = zt_p.tile([128, 16], F32, tag="iz")
                    nc.vector.reciprocal(iz[:], z2[:])
                    ps_zt = psIz.tile([16, 128], F32, tag="ps_zt",
                                      padded_shape=[128, 128])
                    nc.tensor.transpose(ps_zt[:], iz[:], ident[:])
                    nc.scalar.copy(invz_fm[:, t * 128 : (t + 1) * 128], ps_zt[:])

                # ---- numerator + divisor + oh (feature-major, fp8) ----
                oh_all = oh_p.tile([128, 8, 512], FP8, tag="oh_all")
                for c in range(8):
                    ps_n = psNum.tile([128, 512], F32, tag="ps_n")
                    nc.tensor.matmul(
                        ps_n[:], ktq_r[:, c, :], v_sb[:, c, :],
                        start=True, stop=True,
                    )
                    ps_d = psDiv.tile([128, 512], F32, tag="ps_d")
                    nc.tensor.matmul(
                        ps_d[:], s_sel[:, c * 128 : (c + 1) * 128], invz_fm[:],
                        start=True, stop=True,
                    )
                    num_sb = div_p.tile([128, 512], BF16, tag="num_sb")
                    nc.scalar.mul(num_sb[:], ps_n[:], OHS)
                    nc.vector.scalar_tensor_tensor(
                        oh_all[:, c, :], ps_d[:], 1.0, num_sb[:],
                        op0=ALU.mult, op1=ALU.mult,
                    )

                # ---- out-projection + residual + LayerNorm ----
                for t in range(4):
                    s = blk * 4 + t
                    res = xq_tiles[s][:]
                    if bo_bc is not None:
                        qb = lnt.tile([128, D], F32, tag="qb")
                        nc.vector.tensor_tensor(qb[:], res, bo_bc[:], ALU.add)
                        res = qb[:]

                    x_sb = lnt.tile([128, D], BF16, tag="x_sb")
                    s1 = small.tile([128, 2], F32, tag="s1")
                    for of in range(2):
                        sl = slice(of * 512, (of + 1) * 512)
                        ps_o = psO.tile([128, 512], F32, tag="ps_o")
                        for u in range(4):
                            nc.tensor.matmul(
                                ps_o[:],
                                oh_all[:, 2 * u : 2 * u + 2, t * 128 : (t + 1) * 128],
                                wo_t[:, 2 * u : 2 * u + 2, sl],
                                start=(u == 0), stop=(u == 3),
                                perf_mode=DR,
                            )
                        nc.vector.scalar_tensor_tensor(
                            x_sb[:, sl], ps_o[:], 1.0 / (WS * OHS), res[:, sl],
                            op0=ALU.mult, op1=ALU.add,
                            accum_out=s1[:, of : of + 1],
                        )
                    # LN stats: sum via ACT square-accum, mean via matmul-free ops
                    sq = lns.tile([128, D], BF16, tag="sq")
                    s2 = small.tile([128, 1], F32, tag="s2")
                    nc.scalar.activation(sq[:], x_sb[:], AF.Square, accum_out=s2[:])
                    s1t = small.tile([128, 1], F32, tag="s1t")
                    nc.vector.tensor_reduce(
                        s1t[:], s1[:], mybir.AxisListType.X, ALU.add
                    )
                    mu = small.tile([128, 1], F32, tag="mu")
                    nc.scalar.mul(mu[:], s1t[:], 1.0 / D)
                    mu2 = small.tile([128, 1], F32, tag="mu2")
                    nc.scalar.square(mu2[:], mu[:])
                    var = small.tile([128, 1], F32, tag="var")
                    nc.vector.tensor_scalar(
                        var[:], s2[:], 1.0 / D, mu2[:], op0=ALU.mult, op1=ALU.subtract
                    )
                    std = small.tile([128, 1], F32, tag="std")
                    nc.scalar.activation(std[:], var[:], AF.Sqrt, bias=eps_ln[:])
                    rstd = small.tile([128, 1], F32, tag="rstd")
                    nc.vector.reciprocal(rstd[:], std[:])

                    y = lnt.tile([128, D], BF16, tag="y")
                    nc.gpsimd.tensor_scalar(
                        y[:], x_sb[:], mu[:], rstd[:],
                        op0=ALU.subtract, op1=ALU.mult,
                    )
                    if not g_one:
                        nc.vector.tensor_tensor(y[:], y[:], gamma_bc[:], ALU.mult)
                    if not b_zero:
                        nc.vector.tensor_tensor(y[:], y[:], beta_bc[:], ALU.add)
                    nc.sync.dma_start(out[s * 128 : (s + 1) * 128, :], y[:])

    nc.compile()
    return nc


def _get_nc(flags):
    if flags not in _CACHE:
        _CACHE[flags] = _build(*flags)
    return _CACHE[flags]


def _prep(inputs):
    q = np.ascontiguousarray(np.asarray(inputs["query"], dtype=np.float32))
    k = np.ascontiguousarray(np.asarray(inputs["key"], dtype=np.float32))
    v = np.ascontiguousarray(np.asarray(inputs["value"], dtype=np.float32))
    Wq = np.ascontiguousarray(np.asarray(inputs["Wq"], dtype=np.float32))
    Wk = np.ascontiguousarray(np.asarray(inputs["Wk"], dtype=np.float32))
    Wv = np.ascontiguousarray(np.asarray(inputs["Wv"], dtype=np.float32))
    Wo = np.ascontiguousarray(np.asarray(inputs["Wo"], dtype=np.float32))
    bqv = np.ascontiguousarray(np.asarray(inputs["bq"], dtype=np.float32).reshape(1, D))
    bkv = np.ascontiguousarray(np.asarray(inputs["bk"], dtype=np.float32).reshape(1, D))
    bvv = np.ascontiguousarray(np.asarray(inputs["bv"], dtype=np.float32).reshape(1, D))
    bov = np.ascontiguousarray(np.asarray(inputs["bo"], dtype=np.float32).reshape(1, D))
    gv = np.ascontiguousarray(np.asarray(inputs["gamma"], dtype=np.float32).reshape(1, D))
    btv = np.ascontiguousarray(np.asarray(inputs["beta"], dtype=np.float32).reshape(1, D))

    flags = (
        bool(not bqv.any() and not bkv.any()),
        bool(not bvv.any()),
        bool(not bov.any()),
        bool(np.all(gv == 1.0)),
        bool(not btv.any()),
    )
    qf = q.reshape(NCORES, R, D)
    kf = k.reshape(NCORES, R, D)
    vf = v.reshape(NCORES, R, D)
    in_maps = []
    for c in range(NCORES):
        rs = slice(c * 128, (c + 1) * 128)
        in_maps.append(
            {
                "xq": qf[c],
                "xkt": np.ascontiguousarray(kf[c].T),
                "xvt": np.ascontiguousarray(vf[c].T),
                "wq_s": np.ascontiguousarray(Wq[rs]),
                "wk_s": np.ascontiguousarray(Wk[rs]),
                "wv_s": np.ascontiguousarray(Wv[rs]),
                "wo_s": np.ascontiguousarray(Wo[rs]),
                "bq": bqv, "bk": bkv, "bv": bvv, "bo": bov,
                "gamma": gv, "beta": btv,
            }
        )
    return flags, in_maps


def kernel(**inputs):
    from concourse.bass_utils import run_bass_kernel_spmd

    flags, in_maps = _prep(inputs)
    nc = _get_nc(flags)
    res = run_bass_kernel_spmd(nc, in_maps, core_ids=list(range(NCORES)))
    outs = np.stack(
        [np.asarray(res.results[c]["out"], dtype=np.float32) for c in range(NCORES)],
        axis=0,
    )
    return outs.reshape(B, N, D)


# revision 78
# speedup vs baseline: 1.6740x; 1.0182x over previous
"""Trainium2 Bass kernel for linear attention (ELU+1 feature map) block:
Q/K/V projections + linear attention + out-projection + residual + LayerNorm,
distributed over 8 NeuronCores.

Sharding: 8-way row split of (batch*seq); cores 2b, 2b+1 hold the two
2048-token halves of batch b. Per-(batch,head) global reductions (ones-
augmented K^T.Q) are pair-AllReduced. Weights are row-sliced across all 8
cores on the host, cast to fp8 on device, and AllGathered on-chip.

Projections run as fp8e4 DoubleRow matmuls (2 contraction planes per
instruction, 0.5 cyc/row). The feature map is computed as psi = elu(x)
(not elu+1); the ones column of the augmented K matmul reconstructs all
the (psi+1) cross terms after the collective.
"""
import os
import sys

for _p in ("/opt/trn_rl_repo", "/root/.axon_site/_ro/trn_rl_repo"):
    if os.path.isdir(_p) and _p not in sys.path:
        sys.path.insert(0, _p)

import numpy as np

B, N, D, H = 4, 4096, 1024, 16
DEPTH = D // H  # 64
NCORES = 8
R = (B * N) // NCORES  # 2048 rows per core
NSUB = R // 128  # 16 token subtiles per core
NBLK = R // 512  # 4 token blocks in phase B
EPS_Z = 1e-9
EPS_LN = 1e-6
WS = 64.0        # weight pre-scale before fp8 cast
OHS = 16.0       # oh pre-scale before fp8 cast

_CACHE = {}


def _build(zb_qk, zb_v, zb_o, g_one, b_zero, single=False):
    import concourse.bacc as bacc
    import concourse.tile as tile
    from concourse import mybir
    from concourse.masks import make_identity
    from contextlib import ExitStack

    F32 = mybir.dt.float32
    BF16 = mybir.dt.bfloat16
    FP8 = mybir.dt.float8e4
    ALU = mybir.AluOpType
    AF = mybir.ActivationFunctionType
    DR = mybir.MatmulPerfMode.DoubleRow

    nc = bacc.Bacc("TRN2", debug=False, num_devices=1 if single else NCORES)

    xq = nc.dram_tensor("xq", [R, D], F32, kind="ExternalInput").ap()
    xkt = nc.dram_tensor("xkt", [D, R], F32, kind="ExternalInput").ap()
    xvt = nc.dram_tensor("xvt", [D, R], F32, kind="ExternalInput").ap()
    # host-sliced weight rows [c*128:(c+1)*128] of each W
    wq_s = nc.dram_tensor("wq_s", [128, D], F32, kind="ExternalInput").ap()
    wk_s = nc.dram_tensor("wk_s", [128, D], F32, kind="ExternalInput").ap()
    wv_s = nc.dram_tensor("wv_s", [128, D], F32, kind="ExternalInput").ap()
    wo_s = nc.dram_tensor("wo_s", [128, D], F32, kind="ExternalInput").ap()
    bq = nc.dram_tensor("bq", [1, D], F32, kind="ExternalInput").ap()
    bk = nc.dram_tensor("bk", [1, D], F32, kind="ExternalInput").ap()
    bv = nc.dram_tensor("bv", [1, D], F32, kind="ExternalInput").ap()
    bo = nc.dram_tensor("bo", [1, D], F32, kind="ExternalInput").ap()
    gamma = nc.dram_tensor("gamma", [1, D], F32, kind="ExternalInput").ap()
    beta = nc.dram_tensor("beta", [1, D], F32, kind="ExternalInput").ap()
    out = nc.dram_tensor("out", [R, D], BF16, kind="ExternalOutput").ap()

    with tile.TileContext(nc) as tc, ExitStack() as ctx:
        const_p = ctx.enter_context(tc.tile_pool(name="const", bufs=1))
        dp = ctx.enter_context(tc.tile_pool(name="dram", bufs=1, space="DRAM"))
        # long-lived SBUF state
        xq_pool = ctx.enter_context(tc.tile_pool(name="xqn", bufs=1))
        ksb_pool = ctx.enter_context(tc.tile_pool(name="ksb", bufs=1))
        red_pool = ctx.enter_context(tc.tile_pool(name="red", bufs=1))
        wB = ctx.enter_context(tc.tile_pool(name="wB", bufs=1))
        xvt_f_p = ctx.enter_context(tc.tile_pool(name="xvtf", bufs=2))
        xvt_q_p = ctx.enter_context(tc.tile_pool(name="xvtq", bufs=2))

        # ---- weight slice cast + AllGather (first: gates phase A) ----
        cc_w_in = dp.tile([128, 3 * D], FP8, tag="cc_w_in")
        cc_w_out = dp.tile([8 * 128, 3 * D], FP8, tag="cc_w_out")
        cc_wv_in = dp.tile([128, D], BF16, tag="cc_wv_in")
        cc_wv_out = dp.tile([8 * 128, D], BF16, tag="cc_wv_out")
        cc_kq_in = dp.tile([65, D], BF16, tag="cc_kq_in")
        cc_kq_out = dp.tile([65, D], BF16, tag="cc_kq_out")

        s_sel = const_p.tile([16, D], BF16, tag="s_sel")

        from contextlib import ExitStack as _ES0
        with _ES0() as initctx:
            stage = initctx.enter_context(tc.tile_pool(name="stage", bufs=2))
            wslice_q = stage.tile([128, 3 * D], FP8, tag="wslice_q", name="wslice_q")
            for i, w_ap in enumerate((wq_s, wk_s, wo_s)):
                st = stage.tile([128, D], F32, tag="wstage", name=f"wsl_{i}")
                nc.sync.dma_start(st[:], w_ap)
                nc.vector.tensor_scalar_mul(
                    wslice_q[:, i * D : (i + 1) * D], st[:], WS
                )
            nc.sync.dma_start(cc_w_in[:], wslice_q[:])
            # wv slice in bf16 (V path needs the precision; see precsim)
            wv_st = stage.tile([128, D], F32, tag="wstage", name="wsl_v")
            nc.sync.dma_start(wv_st[:], wv_s)
            wv_bf = stage.tile([128, D], BF16, tag="wv_bf", name="wv_bf")
            nc.vector.tensor_copy(wv_bf[:], wv_st[:])
            nc.sync.dma_start(cc_wv_in[:], wv_bf[:])

            if single:
                nc.sync.dma_start(cc_w_out[0:128, :], cc_w_in[:])
                nc.sync.dma_start(cc_wv_out[0:128, :], cc_wv_in[:])
            else:
                nc.gpsimd.collective_compute(
                    "AllGather", ALU.bypass,
                    replica_groups=[list(range(NCORES))],
                    ins=[cc_w_in.opt()], outs=[cc_w_out.opt()],
                )
                nc.gpsimd.collective_compute(
                    "AllGather", ALU.bypass,
                    replica_groups=[list(range(NCORES))],
                    ins=[cc_wv_in.opt()], outs=[cc_wv_out.opt()],
                )

            # S selection matrix (bf16): s_sel[h, f] = 1 iff h == head(f)
            s_f = stage.tile([16, D], F32, tag="wstage", name="s_build",
                             padded_shape=[128, D])
            nc.gpsimd.memset(s_f[:], 0.0)
            s_f3 = s_f[:].rearrange("h (j l) -> h j l", l=64)
            nc.gpsimd.affine_select(
                out=s_f3, in_=s_f3, compare_op=ALU.not_equal, fill=1.0,
                base=0, pattern=[[-1, 16], [0, 64]], channel_multiplier=1,
            )
            nc.vector.tensor_copy(s_sel[:], s_f[:])

        # ---- constants ----
        ident = const_p.tile([128, 128], F32, tag="ident")
        make_identity(nc, ident[:])
        ones_bf = const_p.tile([128, 1], BF16, tag="ones_bf")
        nc.gpsimd.memset(ones_bf[:], 1.0)
        eps_ln = const_p.tile([128, 1], F32, tag="eps_ln")
        nc.gpsimd.memset(eps_ln[:], EPS_LN)

        def bcast_row(name, src_ap):
            row = const_p.tile([1, D], F32, tag=name + "_row")
            nc.sync.dma_start(row[:], src_ap)
            bc = const_p.tile([128, D], F32, tag=name + "_bc")
            nc.gpsimd.partition_broadcast(bc[:], row[:])
            return bc

        bq_bc = None if zb_qk else bcast_row("bq", bq)
        bk_bc = None if zb_qk else bcast_row("bk", bk)
        bo_bc = None if zb_o else bcast_row("bo", bo)
        gamma_bc = None if g_one else bcast_row("gamma", gamma)
        beta_bc = None if b_zero else bcast_row("beta", beta)
        bv_pp = None
        if not zb_v:
            bv_pp = const_p.tile([128, 8], F32, tag="bv_pp")
            for c in range(8):
                nc.sync.dma_start(bv_pp[:, c : c + 1], bv[0:1, c * 128 : (c + 1) * 128])

        # gathered weights -> [128, 8, D] fp8 tiles (k-plane-major pairs)
        cc_w3 = cc_w_out[:].rearrange("(cb p) n -> p cb n", p=128)

        def load_w(pool, widx, name):
            # ACT-queue DMA: waits on the gather without blocking the x
            # prefetch stream on the sync queue
            wt = pool.tile([128, 8, D], FP8, tag=name, name=name)
            nc.scalar.dma_start(wt[:], cc_w3[:, :, widx * D : (widx + 1) * D])
            return wt

        xvt3 = xvt.rearrange("(cb p) m -> p cb m", p=128)

        def load_xvt(blk):
            xvt_q = xvt_q_p.tile([128, 8, 512], BF16, tag="xvt_q",
                                 name=f"xvt_q{blk}")
            for hb in range(4):
                cols = slice(blk * 512 + hb * 128, blk * 512 + (hb + 1) * 128)
                xvt_f = xvt_f_p.tile([128, 8, 128], F32, tag="xvt_f",
                                     name=f"xvt_f{blk}_{hb}")
                nc.sync.dma_start(xvt_f[:], xvt3[:, :, cols])
                nc.gpsimd.tensor_copy(
                    xvt_q[:, :, hb * 128 : (hb + 1) * 128], xvt_f[:]
                )
            return xvt_q

        # ---- persistent activation state ----
        # k_sb[s]: [128 tok, 16 heads, 65] bf16; col 64 = 1.0 (ones augment)
        k_tiles = []
        for s in range(NSUB):
            kt = ksb_pool.tile([128, 16, 65], BF16, tag=f"k_sb{s}", name=f"k_sb{s}")
            nc.gpsimd.memset(kt[:, :, 64:65], 1.0)
            k_tiles.append(kt)
        xq_tiles = [
            xq_pool.tile([128, D], F32, tag=f"xq_nat{s}", name=f"xq_nat{s}")
            for s in range(NSUB)
        ]

        # KtQ rows 0:64 = phiK^T phiQ, row 64 = colsum phiQ (= q_sum)
        ktq_acc = red_pool.tile([65, D], BF16, tag="ktq_acc", padded_shape=[128, D])

        # =========================== PHASE A ===========================
        from contextlib import ExitStack as _ES
        with _ES() as actx:
            wA = actx.enter_context(tc.tile_pool(name="wA", bufs=1))
            wq_t = load_w(wA, 0, "wq_t")
            wk_t = load_w(wA, 1, "wk_t")
            xkt_f_p = actx.enter_context(tc.tile_pool(name="xktf", bufs=2))
            xt_q_p = actx.enter_context(tc.tile_pool(name="xtq", bufs=3))
            qsb_p = actx.enter_context(tc.tile_pool(name="qsb", bufs=3))
            elu_p = actx.enter_context(tc.tile_pool(name="elu", bufs=4))
            psTr = actx.enter_context(tc.tile_pool(name="psTr", bufs=2, space="PSUM"))
            psQK = actx.enter_context(tc.tile_pool(name="psQK", bufs=4, space="PSUM"))
            psKtq = actx.enter_context(tc.tile_pool(name="psKtq", bufs=1, space="PSUM"))

            # KtQ accumulates across all 16 subtiles in 2 resident psum banks:
            # bank0 heads 0-7, bank1 heads 8-15 (cols h*64)
            HB = [(0, 8), (8, 16)]
            kp_banks = []
            for b, (h0, h1) in enumerate(HB):
                kpb = psKtq.tile(
                    [65, (h1 - h0) * 64], F32, tag=f"kp{b}", name=f"kp{b}",
                    padded_shape=[128, (h1 - h0) * 64],
                )
                kp_banks.append(kpb)

            xkt3 = xkt.rearrange("(cb p) m -> p cb m", p=128)

            def project(xt_q, w_t, ps_pool, tag):
                """fp8 DoubleRow projection: out [128 tok, 1024] psum halves."""
                halves = []
                for of in range(2):
                    ph = ps_pool.tile([128, 512], F32, tag="ps_qk", name=f"{tag}_{of}")
                    for u in range(4):
                        nc.tensor.matmul(
                            ph[:],
                            xt_q[:, 2 * u : 2 * u + 2, :],
                            w_t[:, 2 * u : 2 * u + 2, of * 512 : (of + 1) * 512],
                            start=(u == 0), stop=(u == 3),
                            perf_mode=DR,
                        )
                    halves.append(ph)
                return halves

            def phi(halves, dst3, bias_bc, relu_dve):
                """dst3[:, h, 0:64] (bf16) = elu(ps/WS)+1 = relu(ps/WS) + min(e, 1).
                relu on DVE (tensor_scalar) or ACT, to balance the engines."""
                for of in range(2):
                    src = halves[of][:]
                    if bias_bc is not None:
                        xb = elu_p.tile([128, 512], F32, tag="xb")
                        sl = slice(of * 512, (of + 1) * 512)
                        nc.vector.scalar_tensor_tensor(
                            xb[:], src, 1.0 / WS, bias_bc[:, sl],
                            op0=ALU.mult, op1=ALU.add,
                        )
                        src = xb[:]
                        scl = 1.0
                    else:
                        scl = 1.0 / WS
                    e = elu_p.tile([128, 512], BF16, tag="e")
                    nc.scalar.activation(e[:], src, AF.Exp, scale=scl)
                    r = elu_p.tile([128, 512], BF16, tag="r")
                    if relu_dve:
                        nc.vector.tensor_scalar(
                            r[:], src, scl, 0.0, op0=ALU.mult, op1=ALU.max
                        )
                    else:
                        nc.scalar.activation(r[:], src, AF.Relu, scale=scl)
                    dst = dst3[:, of * 8 : (of + 1) * 8, 0:64]
                    nc.vector.scalar_tensor_tensor(
                        dst, e[:], 1.0, r[:], op0=ALU.min, op1=ALU.add
                    )

            for s in range(NSUB):
                rows = slice(s * 128, (s + 1) * 128)
                xq_nat = xq_tiles[s]
                nc.sync.dma_start(xq_nat[:], xq[rows, :])
                xkt_f = xkt_f_p.tile([128, 8, 128], F32, tag="xkt_f")
                nc.sync.dma_start(xkt_f[:], xkt3[:, :, rows])
                xkt_q = xt_q_p.tile([128, 8, 128], FP8, tag="xkt_q")
                nc.gpsimd.tensor_copy(xkt_q[:], xkt_f[:])

                # transpose xq -> fp8 xqt (2 psum banks, 4 transposes each);
                # drains split across ACT/DVE
                xqt_q = xt_q_p.tile([128, 8, 128], FP8, tag="xqt_q")
                for g in range(2):
                    ps_t = psTr.tile([128, 512], F32, tag="trA")
                    for j in range(4):
                        c = g * 4 + j
                        nc.tensor.transpose(
                            ps_t[:, j * 128 : (j + 1) * 128],
                            xq_nat[:, c * 128 : (c + 1) * 128],
                            ident[:],
                        )
                    if g == 0:
                        nc.scalar.copy(xqt_q[:, 0:4, :], ps_t[:])
                    else:
                        nc.vector.tensor_copy(xqt_q[:, 4:8, :], ps_t[:])

                ps_k = project(xkt_q, wk_t, psQK, "ps_k")
                phi(ps_k, k_tiles[s][:], bk_bc, relu_dve=False)

                ps_q = project(xqt_q, wq_t, psQK, "ps_q")
                q_sb = qsb_p.tile([128, 16, 64], BF16, tag="q_sb")
                phi(ps_q, q_sb[:], bq_bc, relu_dve=True)

                # ones-augmented KtQ accumulated in resident psum banks:
                # out[d, (h, e)]: rows 0:64 phiK^T phiQ, row 64 colsum phiQ
                for b, (h0, h1) in enumerate(HB):
                    for h in range(h0, h1):
                        off = (h - h0) * 64
                        nc.tensor.matmul(
                            kp_banks[b][:, off : off + 64],
                            k_tiles[s][:, h, :],
                            q_sb[:, h, :],
                            start=(s == 0), stop=(s == NSUB - 1),
                        )

            # drain the resident KtQ psum banks
            for b, (h0, h1) in enumerate(HB):
                nc.scalar.copy(
                    ktq_acc[:, h0 * 64 : h1 * 64], kp_banks[b][:]
                )

            # prefetch V inputs for the first two phase-B blocks
            wv_t = wB.tile([128, 8, D], BF16, tag="wv_t", name="wv_t")
            nc.scalar.dma_start(
                wv_t[:], cc_wv_out[:].rearrange("(cb p) n -> p cb n", p=128)
            )
            wo_t = load_w(wB, 2, "wo_t")
            xvt_pre = {blk: load_xvt(blk) for blk in range(2)}

        nc.sync.dma_start(cc_kq_in[:], ktq_acc[:])
        if single:
            nc.sync.dma_start(cc_kq_out[:], cc_kq_in[:])
        else:
            nc.gpsimd.collective_compute(
                "AllReduce", ALU.add,
                replica_groups=[[0, 1], [2, 3], [4, 5], [6, 7]],
                ins=[cc_kq_in.opt()], outs=[cc_kq_out.opt()],
            )

        # =========================== PHASE B ===========================
        with _ES() as bctx:
            bred = bctx.enter_context(tc.tile_pool(name="bred", bufs=1))
            vsb_p = bctx.enter_context(tc.tile_pool(name="vsb", bufs=2))
            oh_p = bctx.enter_context(tc.tile_pool(name="oh", bufs=2))
            zt_p = bctx.enter_context(tc.tile_pool(name="zt", bufs=3))
            div_p = bctx.enter_context(tc.tile_pool(name="divsb", bufs=2))
            lnt = bctx.enter_context(tc.tile_pool(name="lnt", bufs=2))
            lns = bctx.enter_context(tc.tile_pool(name="lns", bufs=1))
            small = bctx.enter_context(tc.tile_pool(name="small", bufs=4))
            psV = bctx.enter_context(tc.tile_pool(name="psV", bufs=2, space="PSUM"))
            psNum = bctx.enter_context(tc.tile_pool(name="psNum", bufs=2, space="PSUM"))
            psDiv = bctx.enter_context(tc.tile_pool(name="psDiv", bufs=1, space="PSUM"))
            psO = bctx.enter_context(tc.tile_pool(name="psO", bufs=2, space="PSUM"))
            psIz = bctx.enter_context(tc.tile_pool(name="psIz", bufs=1, space="PSUM"))

            def phase_b_setup():
                """aug rows 0:64 are KtQ (phi cross-products), row 64 is q_sum.
                Build the block-diagonal bf16 lhsT and the q_sum broadcast."""
                qsum_bc = bred.tile([128, D], BF16, tag="qsum_bc")
                ktq_r = bred.tile([128, 8, 128], BF16, tag="ktq_r")
                with _ES() as sctx:
                    tmp = sctx.enter_context(tc.tile_pool(name="pbtmp", bufs=1))
                    rq_bf = tmp.tile([1, D], BF16, tag="rq_bf", padded_shape=[128, D])
                    nc.sync.dma_start(rq_bf[:], cc_kq_out[64:65, :])
                    nc.gpsimd.partition_broadcast(qsum_bc[:], rq_bf[:])

                    aug_bf = tmp.tile([64, D], BF16, tag="aug_bf",
                                      padded_shape=[128, D])
                    nc.sync.dma_start(aug_bf[:], cc_kq_out[0:64, :])
                    nc.gpsimd.memset(ktq_r[:], 0.0)
                    ktv = aug_bf[:].rearrange("d (c t e) -> d c t e", t=2, e=64)
                    nc.sync.dma_start(ktq_r[0:64, :, 0:64], ktv[:, :, 0, :])
                    nc.sync.dma_start(ktq_r[64:128, :, 64:128], ktv[:, :, 1, :])
                return ktq_r, qsum_bc

            ktq_r = qsum_bc = None

            for blk in range(NBLK):
                xvt_q = xvt_pre.pop(blk, None) or load_xvt(blk)
                if blk + 2 < NBLK:
                    xvt_pre[blk + 2] = load_xvt(blk + 2)

                if blk == 0:
                    ktq_r, qsum_bc = phase_b_setup()

                # ---- V projection (feature-major), bf16 for precision ----
                v_sb = vsb_p.tile([128, 8, 512], BF16, tag="v_sb")
                for c in range(8):
                    ps_v = psV.tile([128, 512], F32, tag="ps_v")
                    for u in range(8):
                        nc.tensor.matmul(
                            ps_v[:],
                            wv_t[:, u, c * 128 : (c + 1) * 128],
                            xvt_q[:, u, :],
                            start=(u == 0), stop=(u == 7),
                        )
                    if zb_v:
                        nc.scalar.copy(v_sb[:, c, :], ps_v[:])
                    else:
                        nc.scalar.activation(
                            v_sb[:, c, :], ps_v[:], AF.Identity,
                            bias=bv_pp[:, c : c + 1],
                        )

                # ---- z = psiK . qsum_phi + S (token-major), invz -> feat-major
                invz_fm = div_p.tile([16, 512], BF16, tag="invz_fm",
                                     padded_shape=[128, 512])
                for t in range(4):
                    s = blk * 4 + t
                    prod = zt_p.tile([128, 16, 64], BF16, tag="prod")
                    eng = nc.gpsimd if t % 2 == 0 else nc.vector
                    eng.tensor_tensor(
                        prod[:], k_tiles[s][:, :, 0:64],
                        qsum_bc[:].rearrange("p (h e) -> p h e", e=64),
                        ALU.mult,
                    )
                    zraw = zt_p.tile([128, 16], F32, tag="zraw")
                    nc.vector.tensor_reduce(
                        zraw[:], prod[:], mybir.AxisListType.X, ALU.add
                    )
                    z2 = zt_p.tile([128, 16], F32, tag="z2")
                    nc.vector.tensor_scalar_add(z2[:], zraw[:], EPS_Z)
                    iz = zt_p.tile([128, 16], F32, tag="iz")
                    nc.vector.reciprocal(iz[:], z2[:])
                    ps_zt = psIz.tile([16, 128], F32, tag="ps_zt",
                                      padded_shape=[128, 128])
                    nc.tensor.transpose(ps_zt[:], iz[:], ident[:])
                    nc.scalar.copy(invz_fm[:, t * 128 : (t + 1) * 128], ps_zt[:])

                # ---- numerator + divisor + oh (feature-major, fp8) ----
                oh_all = oh_p.tile([128, 8, 512], FP8, tag="oh_all")
                for c in range(8):
                    ps_n = psNum.tile([128, 512], F32, tag="ps_n")
                    nc.tensor.matmul(
                        ps_n[:], ktq_r[:, c, :], v_sb[:, c, :],
                        start=True, stop=True,
                    )
                    ps_d = psDiv.tile([128, 512], F32, tag="ps_d")
                    nc.tensor.matmul(
                        ps_d[:], s_sel[:, c * 128 : (c + 1) * 128], invz_fm[:],
                        start=True, stop=True,
                    )
                    num_sb = div_p.tile([128, 512], BF16, tag="num_sb")
                    nc.scalar.mul(num_sb[:], ps_n[:], OHS)
                    nc.vector.scalar_tensor_tensor(
                        oh_all[:, c, :], ps_d[:], 1.0, num_sb[:],
                        op0=ALU.mult, op1=ALU.mult,
                    )

                # ---- out-projection + residual + LayerNorm ----
                for t in range(4):
                    s = blk * 4 + t
                    res = xq_tiles[s][:]
                    if bo_bc is not None:
                        qb = lnt.tile([128, D], F32, tag="qb")
                        nc.vector.tensor_tensor(qb[:], res, bo_bc[:], ALU.add)
                        res = qb[:]

                    x_sb = lnt.tile([128, D], BF16, tag="x_sb")
                    s1 = small.tile([128, 2], F32, tag="s1")
                    for of in range(2):
                        sl = slice(of * 512, (of + 1) * 512)
                        ps_o = psO.tile([128, 512], F32, tag="ps_o")
                        for u in range(4):
                            nc.tensor.matmul(
                                ps_o[:],
                                oh_all[:, 2 * u : 2 * u + 2, t * 128 : (t + 1) * 128],
                                wo_t[:, 2 * u : 2 * u + 2, sl],
                                start=(u == 0), stop=(u == 3),
                                perf_mode=DR,
                            )
                        nc.vector.scalar_tensor_tensor(
                            x_sb[:, sl], ps_o[:], 1.0 / (WS * OHS), res[:, sl],
                            op0=ALU.mult, op1=ALU.add,
                            accum_out=s1[:, of : of + 1],
                        )
                    # LN stats: sum via ACT square-accum, mean via matmul-free ops
                    sq = lns.tile([128, D], BF16, tag="sq")
                    s2 = small.tile([128, 1], F32, tag="s2")
                    nc.scalar.activation(sq[:], x_sb[:], AF.Square, accum_out=s2[:])
                    s1t = small.tile([128, 1], F32, tag="s1t")
                    nc.vector.tensor_reduce(
                        s1t[:], s1[:], mybir.AxisListType.X, ALU.add
                    )
                    mu = small.tile([128, 1], F32, tag="mu")
                    nc.scalar.mul(mu[:], s1t[:], 1.0 / D)
                    mu2 = small.tile([128, 1], F32, tag="mu2")
                    nc.scalar.square(mu2[:], mu[:])
                    var = small.tile([128, 1], F32, tag="var")
                    nc.vector.tensor_scalar(
                        var[:], s2[:], 1.0 / D, mu2[:], op0=ALU.mult, op1=ALU.subtract
                    )
                    std = small.tile([128, 1], F32, tag="std")
                    nc.scalar.activation(std[:], var[:], AF.Sqrt, bias=eps_ln[:])
                    rstd = small.tile([128, 1], F32, tag="rstd")
                    nc.vector.reciprocal(rstd[:], std[:])

                    y = lnt.tile([128, D], BF16, tag="y")
                    yeng = nc.gpsimd if t % 2 == 0 else nc.vector
                    yeng.tensor_scalar(
                        y[:], x_sb[:], mu[:], rstd[:],
                        op0=ALU.subtract, op1=ALU.mult,
                    )
                    if not g_one:
                        nc.vector.tensor_tensor(y[:], y[:], gamma_bc[:], ALU.mult)
                    if not b_zero:
                        nc.vector.tensor_tensor(y[:], y[:], beta_bc[:], ALU.add)
                    nc.sync.dma_start(out[s * 128 : (s + 1) * 128, :], y[:])

    nc.compile()
    return nc


def _get_nc(flags):
    if flags not in _CACHE:
        _CACHE[flags] = _build(*flags)
    return _CACHE[flags]


def _prep(inputs):
    q = np.ascontiguousarray(np.asarray(inputs["query"], dtype=np.float32))
    k = np.ascontiguousarray(np.asarray(inputs["key"], dtype=np.float32))
    v = np.ascontiguousarray(np.asarray(inputs["value"], dtype=np.float32))
    Wq = np.ascontiguousarray(np.asarray(inputs["Wq"], dtype=np.float32))
    Wk = np.ascontiguousarray(np.asarray(inputs["Wk"], dtype=np.float32))
    Wv = np.ascontiguousarray(np.asarray(inputs["Wv"], dtype=np.float32))
    Wo = np.ascontiguousarray(np.asarray(inputs["Wo"], dtype=np.float32))
    bqv = np.ascontiguousarray(np.asarray(inputs["bq"], dtype=np.float32).reshape(1, D))
    bkv = np.ascontiguousarray(np.asarray(inputs["bk"], dtype=np.float32).reshape(1, D))
    bvv = np.ascontiguousarray(np.asarray(inputs["bv"], dtype=np.float32).reshape(1, D))
    bov = np.ascontiguousarray(np.asarray(inputs["bo"], dtype=np.float32).reshape(1, D))
    gv = np.ascontiguousarray(np.asarray(inputs["gamma"], dtype=np.float32).reshape(1, D))
    btv = np.ascontiguousarray(np.asarray(inputs["beta"], dtype=np.float32).reshape(1, D))

    flags = (
        bool(not bqv.any() and not bkv.any()),
        bool(not bvv.any()),
        bool(not bov.any()),
        bool(np.all(gv == 1.0)),
        bool(not btv.any()),
    )
    qf = q.reshape(NCORES, R, D)
    kf = k.reshape(NCORES, R, D)
    vf = v.reshape(NCORES, R, D)
    in_maps = []
    for c in range(NCORES):
        rs = slice(c * 128, (c + 1) * 128)
        in_maps.append(
            {
                "xq": qf[c],
                "xkt": np.ascontiguousarray(kf[c].T),
                "xvt": np.ascontiguousarray(vf[c].T),
                "wq_s": np.ascontiguousarray(Wq[rs]),
                "wk_s": np.ascontiguousarray(Wk[rs]),
                "wv_s": np.ascontiguousarray(Wv[rs]),
                "wo_s": np.ascontiguousarray(Wo[rs]),
                "bq": bqv, "bk": bkv, "bv": bvv, "bo": bov,
                "gamma": gv, "beta": btv,
            }
        )
    return flags, in_maps


def kernel(**inputs):
    from concourse.bass_utils import run_bass_kernel_spmd

    flags, in_maps = _prep(inputs)
    nc = _get_nc(flags)
    res = run_bass_kernel_spmd(nc, in_maps, core_ids=list(range(NCORES)))
    outs = np.stack(
        [np.asarray(res.results[c]["out"], dtype=np.float32) for c in range(NCORES)],
        axis=0,
    )
    return outs.reshape(B, N, D)


# revision 83
# speedup vs baseline: 1.7390x; 1.0389x over previous
"""Trainium2 Bass kernel for linear attention (ELU+1 feature map) block:
Q/K/V projections + linear attention + out-projection + residual + LayerNorm,
distributed over 8 NeuronCores.

Sharding: 8-way row split of (batch*seq); cores 2b, 2b+1 hold the two
2048-token halves of batch b. Per-(batch,head) global reductions (ones-
augmented K^T.Q) are pair-AllReduced. Weights are row-sliced across all 8
cores on the host, cast to fp8 on device, and AllGathered on-chip.

Projections run as fp8e4 DoubleRow matmuls (2 contraction planes per
instruction, 0.5 cyc/row). The feature map is computed as psi = elu(x)
(not elu+1); the ones column of the augmented K matmul reconstructs all
the (psi+1) cross terms after the collective.
"""
import os
import sys

for _p in ("/opt/trn_rl_repo", "/root/.axon_site/_ro/trn_rl_repo"):
    if os.path.isdir(_p) and _p not in sys.path:
        sys.path.insert(0, _p)

import numpy as np

B, N, D, H = 4, 4096, 1024, 16
DEPTH = D // H  # 64
NCORES = 8
R = (B * N) // NCORES  # 2048 rows per core
NSUB = R // 128  # 16 token subtiles per core
NBLK = R // 512  # 4 token blocks in phase B
EPS_Z = 1e-9
EPS_LN = 1e-6
WS = 64.0        # weight pre-scale before fp8 cast
OHS = 16.0       # oh pre-scale before fp8 cast

_CACHE = {}


def _build(zb_qk, zb_v, zb_o, g_one, b_zero, single=False):
    import concourse.bacc as bacc
    import concourse.tile as tile
    from concourse import mybir
    from concourse.masks import make_identity
    from contextlib import ExitStack

    F32 = mybir.dt.float32
    BF16 = mybir.dt.bfloat16
    FP8 = mybir.dt.float8e4
    ALU = mybir.AluOpType
    AF = mybir.ActivationFunctionType
    DR = mybir.MatmulPerfMode.DoubleRow

    nc = bacc.Bacc("TRN2", debug=False, num_devices=1 if single else NCORES)

    xq = nc.dram_tensor("xq", [R, D], F32, kind="ExternalInput").ap()
    xkt = nc.dram_tensor("xkt", [D, R], F32, kind="ExternalInput").ap()
    xvt = nc.dram_tensor("xvt", [D, R], F32, kind="ExternalInput").ap()
    # host-sliced weight rows [c*128:(c+1)*128] of each W
    wq_s = nc.dram_tensor("wq_s", [128, D], F32, kind="ExternalInput").ap()
    wk_s = nc.dram_tensor("wk_s", [128, D], F32, kind="ExternalInput").ap()
    wv_s = nc.dram_tensor("wv_s", [128, D], F32, kind="ExternalInput").ap()
    wo_s = nc.dram_tensor("wo_s", [128, D], F32, kind="ExternalInput").ap()
    bq = nc.dram_tensor("bq", [1, D], F32, kind="ExternalInput").ap()
    bk = nc.dram_tensor("bk", [1, D], F32, kind="ExternalInput").ap()
    bv = nc.dram_tensor("bv", [1, D], F32, kind="ExternalInput").ap()
    bo = nc.dram_tensor("bo", [1, D], F32, kind="ExternalInput").ap()
    gamma = nc.dram_tensor("gamma", [1, D], F32, kind="ExternalInput").ap()
    beta = nc.dram_tensor("beta", [1, D], F32, kind="ExternalInput").ap()
    out = nc.dram_tensor("out", [R, D], BF16, kind="ExternalOutput").ap()

    with tile.TileContext(nc) as tc, ExitStack() as ctx:
        const_p = ctx.enter_context(tc.tile_pool(name="const", bufs=1))
        dp = ctx.enter_context(tc.tile_pool(name="dram", bufs=1, space="DRAM"))
        # long-lived SBUF state
        xq_pool = ctx.enter_context(tc.tile_pool(name="xqn", bufs=1))
        ksb_pool = ctx.enter_context(tc.tile_pool(name="ksb", bufs=1))
        red_pool = ctx.enter_context(tc.tile_pool(name="red", bufs=1))
        wB = ctx.enter_context(tc.tile_pool(name="wB", bufs=1))
        xvt_f_p = ctx.enter_context(tc.tile_pool(name="xvtf", bufs=2))
        xvt_q_p = ctx.enter_context(tc.tile_pool(name="xvtq", bufs=2))

        # ---- weight slice cast + AllGather (first: gates phase A) ----
        cc_w_in = dp.tile([128, 3 * D], FP8, tag="cc_w_in")
        cc_w_out = dp.tile([8 * 128, 3 * D], FP8, tag="cc_w_out")
        cc_wv_in = dp.tile([128, D], BF16, tag="cc_wv_in")
        cc_wv_out = dp.tile([8 * 128, D], BF16, tag="cc_wv_out")
        cc_kq_in = dp.tile([65, D], BF16, tag="cc_kq_in")
        cc_kq_out = dp.tile([65, D], BF16, tag="cc_kq_out")

        s_sel = const_p.tile([16, D], BF16, tag="s_sel")

        from contextlib import ExitStack as _ES0
        with _ES0() as initctx:
            stage = initctx.enter_context(tc.tile_pool(name="stage", bufs=2))
            wslice_q = stage.tile([128, 3 * D], FP8, tag="wslice_q", name="wslice_q")
            for i, w_ap in enumerate((wq_s, wk_s, wo_s)):
                st = stage.tile([128, D], F32, tag="wstage", name=f"wsl_{i}")
                nc.sync.dma_start(st[:], w_ap)
                nc.vector.tensor_scalar_mul(
                    wslice_q[:, i * D : (i + 1) * D], st[:], WS
                )
            nc.sync.dma_start(cc_w_in[:], wslice_q[:])
            # wv slice in bf16 (V path needs the precision; see precsim)
            wv_st = stage.tile([128, D], F32, tag="wstage", name="wsl_v")
            nc.sync.dma_start(wv_st[:], wv_s)
            wv_bf = stage.tile([128, D], BF16, tag="wv_bf", name="wv_bf")
            nc.vector.tensor_copy(wv_bf[:], wv_st[:])
            nc.sync.dma_start(cc_wv_in[:], wv_bf[:])

            if single:
                nc.sync.dma_start(cc_w_out[0:128, :], cc_w_in[:])
                nc.sync.dma_start(cc_wv_out[0:128, :], cc_wv_in[:])
            else:
                nc.gpsimd.collective_compute(
                    "AllGather", ALU.bypass,
                    replica_groups=[list(range(NCORES))],
                    ins=[cc_w_in.opt()], outs=[cc_w_out.opt()],
                )
                nc.gpsimd.collective_compute(
                    "AllGather", ALU.bypass,
                    replica_groups=[list(range(NCORES))],
                    ins=[cc_wv_in.opt()], outs=[cc_wv_out.opt()],
                )

            # S selection matrix (bf16): s_sel[h, f] = 1 iff h == head(f)
            s_f = stage.tile([16, D], F32, tag="wstage", name="s_build",
                             padded_shape=[128, D])
            nc.gpsimd.memset(s_f[:], 0.0)
            s_f3 = s_f[:].rearrange("h (j l) -> h j l", l=64)
            nc.gpsimd.affine_select(
                out=s_f3, in_=s_f3, compare_op=ALU.not_equal, fill=1.0,
                base=0, pattern=[[-1, 16], [0, 64]], channel_multiplier=1,
            )
            nc.vector.tensor_copy(s_sel[:], s_f[:])

        # ---- constants ----
        ident = const_p.tile([128, 128], F32, tag="ident")
        make_identity(nc, ident[:])
        ones_bf = const_p.tile([128, 1], BF16, tag="ones_bf")
        nc.gpsimd.memset(ones_bf[:], 1.0)
        eps_ln = const_p.tile([128, 1], F32, tag="eps_ln")
        nc.gpsimd.memset(eps_ln[:], EPS_LN)

        def bcast_row(name, src_ap):
            row = const_p.tile([1, D], F32, tag=name + "_row")
            nc.sync.dma_start(row[:], src_ap)
            bc = const_p.tile([128, D], F32, tag=name + "_bc")
            nc.gpsimd.partition_broadcast(bc[:], row[:])
            return bc

        bq_bc = None if zb_qk else bcast_row("bq", bq)
        bk_bc = None if zb_qk else bcast_row("bk", bk)
        bo_bc = None if zb_o else bcast_row("bo", bo)
        gamma_bc = None if g_one else bcast_row("gamma", gamma)
        beta_bc = None if b_zero else bcast_row("beta", beta)
        bv_pp = None
        if not zb_v:
            bv_pp = const_p.tile([128, 8], F32, tag="bv_pp")
            for c in range(8):
                nc.sync.dma_start(bv_pp[:, c : c + 1], bv[0:1, c * 128 : (c + 1) * 128])

        # gathered weights -> [128, 8, D] fp8 tiles (k-plane-major pairs)
        cc_w3 = cc_w_out[:].rearrange("(cb p) n -> p cb n", p=128)

        def load_w(pool, widx, name):
            # ACT-queue DMA: waits on the gather without blocking the x
            # prefetch stream on the sync queue
            wt = pool.tile([128, 8, D], FP8, tag=name, name=name)
            nc.scalar.dma_start(wt[:], cc_w3[:, :, widx * D : (widx + 1) * D])
            return wt

        xvt3 = xvt.rearrange("(cb p) m -> p cb m", p=128)

        def load_xvt(blk):
            xvt_q = xvt_q_p.tile([128, 8, 512], BF16, tag="xvt_q",
                                 name=f"xvt_q{blk}")
            for hb in range(4):
                cols = slice(blk * 512 + hb * 128, blk * 512 + (hb + 1) * 128)
                xvt_f = xvt_f_p.tile([128, 8, 128], F32, tag="xvt_f",
                                     name=f"xvt_f{blk}_{hb}")
                nc.sync.dma_start(xvt_f[:], xvt3[:, :, cols])
                nc.gpsimd.tensor_copy(
                    xvt_q[:, :, hb * 128 : (hb + 1) * 128], xvt_f[:]
                )
            return xvt_q

        # ---- persistent activation state ----
        # k_sb[s]: [128 tok, 16 heads, 65] bf16; col 64 = 1.0 (ones augment)
        k_tiles = []
        for s in range(NSUB):
            kt = ksb_pool.tile([128, 16, 65], BF16, tag=f"k_sb{s}", name=f"k_sb{s}")
            nc.gpsimd.memset(kt[:, :, 64:65], 1.0)
            k_tiles.append(kt)
        xq_tiles = [
            xq_pool.tile([128, D], F32, tag=f"xq_nat{s}", name=f"xq_nat{s}")
            for s in range(NSUB)
        ]

        # KtQ rows 0:64 = phiK^T phiQ, row 64 = colsum phiQ (= q_sum)
        ktq_acc = red_pool.tile([65, D], BF16, tag="ktq_acc", padded_shape=[128, D])

        # =========================== PHASE A ===========================
        from contextlib import ExitStack as _ES
        with _ES() as actx:
            wA = actx.enter_context(tc.tile_pool(name="wA", bufs=1))
            wk_t = load_w(wA, 1, "wk_t")
            wq_t = wA.tile([128, 8, D], FP8, tag="wq_t", name="wq_t")
            nc.gpsimd.dma_start(wq_t[:], cc_w3[:, :, 0:D])
            xkt_f_p = actx.enter_context(tc.tile_pool(name="xktf", bufs=2))
            xt_q_p = actx.enter_context(tc.tile_pool(name="xtq", bufs=3))
            qsb_p = actx.enter_context(tc.tile_pool(name="qsb", bufs=3))
            elu_p = actx.enter_context(tc.tile_pool(name="elu", bufs=4))
            psTr = actx.enter_context(tc.tile_pool(name="psTr", bufs=2, space="PSUM"))
            psQK = actx.enter_context(tc.tile_pool(name="psQK", bufs=4, space="PSUM"))
            psKtq = actx.enter_context(tc.tile_pool(name="psKtq", bufs=1, space="PSUM"))

            # KtQ accumulates across all 16 subtiles in 2 resident psum banks:
            # bank0 heads 0-7, bank1 heads 8-15 (cols h*64)
            HB = [(0, 8), (8, 16)]
            kp_banks = []
            for b, (h0, h1) in enumerate(HB):
                kpb = psKtq.tile(
                    [65, (h1 - h0) * 64], F32, tag=f"kp{b}", name=f"kp{b}",
                    padded_shape=[128, (h1 - h0) * 64],
                )
                kp_banks.append(kpb)

            xkt3 = xkt.rearrange("(cb p) m -> p cb m", p=128)

            def project(xt_q, w_t, ps_pool, tag):
                """fp8 DoubleRow projection: out [128 tok, 1024] psum halves."""
                halves = []
                for of in range(2):
                    ph = ps_pool.tile([128, 512], F32, tag="ps_qk", name=f"{tag}_{of}")
                    for u in range(4):
                        nc.tensor.matmul(
                            ph[:],
                            xt_q[:, 2 * u : 2 * u + 2, :],
                            w_t[:, 2 * u : 2 * u + 2, of * 512 : (of + 1) * 512],
                            start=(u == 0), stop=(u == 3),
                            perf_mode=DR,
                        )
                    halves.append(ph)
                return halves

            def phi(halves, dst3, bias_bc, relu_dve):
                """dst3[:, h, 0:64] (bf16) = elu(ps/WS)+1 = relu(ps/WS) + min(e, 1).
                relu on DVE (tensor_scalar) or ACT, to balance the engines."""
                for of in range(2):
                    src = halves[of][:]
                    if bias_bc is not None:
                        xb = elu_p.tile([128, 512], F32, tag="xb")
                        sl = slice(of * 512, (of + 1) * 512)
                        nc.vector.scalar_tensor_tensor(
                            xb[:], src, 1.0 / WS, bias_bc[:, sl],
                            op0=ALU.mult, op1=ALU.add,
                        )
                        src = xb[:]
                        scl = 1.0
                    else:
                        scl = 1.0 / WS
                    e = elu_p.tile([128, 512], BF16, tag="e")
                    nc.scalar.activation(e[:], src, AF.Exp, scale=scl)
                    r = elu_p.tile([128, 512], BF16, tag="r")
                    if relu_dve:
                        nc.vector.tensor_scalar(
                            r[:], src, scl, 0.0, op0=ALU.mult, op1=ALU.max
                        )
                    else:
                        nc.scalar.activation(r[:], src, AF.Relu, scale=scl)
                    dst = dst3[:, of * 8 : (of + 1) * 8, 0:64]
                    nc.vector.scalar_tensor_tensor(
                        dst, e[:], 1.0, r[:], op0=ALU.min, op1=ALU.add
                    )

            for s in range(NSUB):
                rows = slice(s * 128, (s + 1) * 128)
                xq_nat = xq_tiles[s]
                nc.sync.dma_start(xq_nat[:], xq[rows, :])
                xkt_f = xkt_f_p.tile([128, 8, 128], F32, tag="xkt_f")
                nc.sync.dma_start(xkt_f[:], xkt3[:, :, rows])
                xkt_q = xt_q_p.tile([128, 8, 128], FP8, tag="xkt_q")
                nc.gpsimd.tensor_copy(xkt_q[:], xkt_f[:])

                # transpose xq -> fp8 xqt (2 psum banks, 4 transposes each);
                # drains split across ACT/DVE
                xqt_q = xt_q_p.tile([128, 8, 128], FP8, tag="xqt_q")
                for g in range(2):
                    ps_t = psTr.tile([128, 512], F32, tag="trA")
                    for j in range(4):
                        c = g * 4 + j
                        nc.tensor.transpose(
                            ps_t[:, j * 128 : (j + 1) * 128],
                            xq_nat[:, c * 128 : (c + 1) * 128],
                            ident[:],
                        )
                    if g == 0:
                        nc.scalar.copy(xqt_q[:, 0:4, :], ps_t[:])
                    else:
                        nc.vector.tensor_copy(xqt_q[:, 4:8, :], ps_t[:])

                ps_k = project(xkt_q, wk_t, psQK, "ps_k")
                phi(ps_k, k_tiles[s][:], bk_bc, relu_dve=False)

                ps_q = project(xqt_q, wq_t, psQK, "ps_q")
                q_sb = qsb_p.tile([128, 16, 64], BF16, tag="q_sb")
                phi(ps_q, q_sb[:], bq_bc, relu_dve=True)

                # ones-augmented KtQ accumulated in resident psum banks:
                # out[d, (h, e)]: rows 0:64 phiK^T phiQ, row 64 colsum phiQ
                for b, (h0, h1) in enumerate(HB):
                    for h in range(h0, h1):
                        off = (h - h0) * 64
                        nc.tensor.matmul(
                            kp_banks[b][:, off : off + 64],
                            k_tiles[s][:, h, :],
                            q_sb[:, h, :],
                            start=(s == 0), stop=(s == NSUB - 1),
                        )

            # drain the resident KtQ psum banks
            for b, (h0, h1) in enumerate(HB):
                nc.scalar.copy(
                    ktq_acc[:, h0 * 64 : h1 * 64], kp_banks[b][:]
                )

            # prefetch V inputs for the first two phase-B blocks
            wv_t = wB.tile([128, 8, D], BF16, tag="wv_t", name="wv_t")
            nc.scalar.dma_start(
                wv_t[:], cc_wv_out[:].rearrange("(cb p) n -> p cb n", p=128)
            )
            wo_t = load_w(wB, 2, "wo_t")
            xvt_pre = {blk: load_xvt(blk) for blk in range(2)}

        nc.sync.dma_start(cc_kq_in[:], ktq_acc[:])
        if single:
            nc.sync.dma_start(cc_kq_out[:], cc_kq_in[:])
        else:
            nc.gpsimd.collective_compute(
                "AllReduce", ALU.add,
                replica_groups=[[0, 1], [2, 3], [4, 5], [6, 7]],
                ins=[cc_kq_in.opt()], outs=[cc_kq_out.opt()],
            )

        # =========================== PHASE B ===========================
        with _ES() as bctx:
            bred = bctx.enter_context(tc.tile_pool(name="bred", bufs=1))
            vsb_p = bctx.enter_context(tc.tile_pool(name="vsb", bufs=2))
            oh_p = bctx.enter_context(tc.tile_pool(name="oh", bufs=2))
            zt_p = bctx.enter_context(tc.tile_pool(name="zt", bufs=3))
            div_p = bctx.enter_context(tc.tile_pool(name="divsb", bufs=2))
            lnt = bctx.enter_context(tc.tile_pool(name="lnt", bufs=2))
            lns = bctx.enter_context(tc.tile_pool(name="lns", bufs=1))
            small = bctx.enter_context(tc.tile_pool(name="small", bufs=4))
            psV = bctx.enter_context(tc.tile_pool(name="psV", bufs=2, space="PSUM"))
            psNum = bctx.enter_context(tc.tile_pool(name="psNum", bufs=2, space="PSUM"))
            psDiv = bctx.enter_context(tc.tile_pool(name="psDiv", bufs=1, space="PSUM"))
            psO = bctx.enter_context(tc.tile_pool(name="psO", bufs=2, space="PSUM"))
            psIz = bctx.enter_context(tc.tile_pool(name="psIz", bufs=1, space="PSUM"))

            def phase_b_setup():
                """aug rows 0:64 are KtQ (phi cross-products), row 64 is q_sum.
                Build the block-diagonal bf16 lhsT and the q_sum broadcast."""
                qsum_bc = bred.tile([128, D], BF16, tag="qsum_bc")
                ktq_r = bred.tile([128, 8, 128], BF16, tag="ktq_r")
                with _ES() as sctx:
                    tmp = sctx.enter_context(tc.tile_pool(name="pbtmp", bufs=1))
                    rq_bf = tmp.tile([1, D], BF16, tag="rq_bf", padded_shape=[128, D])
                    nc.sync.dma_start(rq_bf[:], cc_kq_out[64:65, :])
                    nc.gpsimd.partition_broadcast(qsum_bc[:], rq_bf[:])

                    aug_bf = tmp.tile([64, D], BF16, tag="aug_bf",
                                      padded_shape=[128, D])
                    nc.sync.dma_start(aug_bf[:], cc_kq_out[0:64, :])
                    nc.gpsimd.memset(ktq_r[:], 0.0)
                    ktv = aug_bf[:].rearrange("d (c t e) -> d c t e", t=2, e=64)
                    nc.sync.dma_start(ktq_r[0:64, :, 0:64], ktv[:, :, 0, :])
                    nc.sync.dma_start(ktq_r[64:128, :, 64:128], ktv[:, :, 1, :])
                return ktq_r, qsum_bc

            ktq_r = qsum_bc = None

            for blk in range(NBLK):
                xvt_q = xvt_pre.pop(blk, None) or load_xvt(blk)
                if blk + 2 < NBLK:
                    xvt_pre[blk + 2] = load_xvt(blk + 2)

                if blk == 0:
                    ktq_r, qsum_bc = phase_b_setup()

                # ---- V projection (feature-major), bf16 for precision ----
                v_sb = vsb_p.tile([128, 8, 512], BF16, tag="v_sb")
                for c in range(8):
                    ps_v = psV.tile([128, 512], F32, tag="ps_v")
                    for u in range(8):
                        nc.tensor.matmul(
                            ps_v[:],
                            wv_t[:, u, c * 128 : (c + 1) * 128],
                            xvt_q[:, u, :],
                            start=(u == 0), stop=(u == 7),
                        )
                    if zb_v:
                        nc.scalar.copy(v_sb[:, c, :], ps_v[:])
                    else:
                        nc.scalar.activation(
                            v_sb[:, c, :], ps_v[:], AF.Identity,
                            bias=bv_pp[:, c : c + 1],
                        )

                # ---- z = psiK . qsum_phi + S (token-major), invz -> feat-major
                invz_fm = div_p.tile([16, 512], BF16, tag="invz_fm",
                                     padded_shape=[128, 512])
                for t in range(4):
                    s = blk * 4 + t
                    prod = zt_p.tile([128, 16, 64], BF16, tag="prod")
                    eng = nc.gpsimd if t % 2 == 0 else nc.vector
                    eng.tensor_tensor(
                        prod[:], k_tiles[s][:, :, 0:64],
                        qsum_bc[:].rearrange("p (h e) -> p h e", e=64),
                        ALU.mult,
                    )
                    zraw = zt_p.tile([128, 16], F32, tag="zraw")
                    nc.vector.tensor_reduce(
                        zraw[:], prod[:], mybir.AxisListType.X, ALU.add
                    )
                    z2 = zt_p.tile([128, 16], F32, tag="z2")
                    nc.vector.tensor_scalar_add(z2[:], zraw[:], EPS_Z)
                    iz = zt_p.tile([128, 16], F32, tag="iz")
                    nc.vector.reciprocal(iz[:], z2[:])
                    ps_zt = psIz.tile([16, 128], F32, tag="ps_zt",
                                      padded_shape=[128, 128])
                    nc.tensor.transpose(ps_zt[:], iz[:], ident[:])
                    nc.scalar.copy(invz_fm[:, t * 128 : (t + 1) * 128], ps_zt[:])

                # ---- numerator + divisor + oh (feature-major, fp8) ----
                oh_all = oh_p.tile([128, 8, 512], FP8, tag="oh_all")
                for c in range(8):
                    ps_n = psNum.tile([128, 512], F32, tag="ps_n")
                    nc.tensor.matmul(
                        ps_n[:], ktq_r[:, c, :], v_sb[:, c, :],
                        start=True, stop=True,
                    )
                    ps_d = psDiv.tile([128, 512], F32, tag="ps_d")
                    nc.tensor.matmul(
                        ps_d[:], s_sel[:, c * 128 : (c + 1) * 128], invz_fm[:],
                        start=True, stop=True,
                    )
                    num_sb = div_p.tile([128, 512], BF16, tag="num_sb")
                    nc.scalar.mul(num_sb[:], ps_n[:], OHS)
                    nc.vector.scalar_tensor_tensor(
                        oh_all[:, c, :], ps_d[:], 1.0, num_sb[:],
                        op0=ALU.mult, op1=ALU.mult,
                    )

                # ---- out-projection + residual + LayerNorm ----
                for t in range(4):
                    s = blk * 4 + t
                    res = xq_tiles[s][:]
                    if bo_bc is not None:
                        qb = lnt.tile([128, D], F32, tag="qb")
                        nc.vector.tensor_tensor(qb[:], res, bo_bc[:], ALU.add)
                        res = qb[:]

                    x_sb = lnt.tile([128, D], BF16, tag="x_sb")
                    s1 = small.tile([128, 2], F32, tag="s1")
                    for of in range(2):
                        sl = slice(of * 512, (of + 1) * 512)
                        ps_o = psO.tile([128, 512], F32, tag="ps_o")
                        for u in range(4):
                            nc.tensor.matmul(
                                ps_o[:],
                                oh_all[:, 2 * u : 2 * u + 2, t * 128 : (t + 1) * 128],
                                wo_t[:, 2 * u : 2 * u + 2, sl],
                                start=(u == 0), stop=(u == 3),
                                perf_mode=DR,
                            )
                        nc.vector.scalar_tensor_tensor(
                            x_sb[:, sl], ps_o[:], 1.0 / (WS * OHS), res[:, sl],
                            op0=ALU.mult, op1=ALU.add,
                            accum_out=s1[:, of : of + 1],
                        )
                    # LN stats: sum via ACT square-accum, mean via matmul-free ops
                    sq = lns.tile([128, D], BF16, tag="sq")
                    s2 = small.tile([128, 1], F32, tag="s2")
                    nc.scalar.activation(sq[:], x_sb[:], AF.Square, accum_out=s2[:])
                    s1t = small.tile([128, 1], F32, tag="s1t")
                    nc.vector.tensor_reduce(
                        s1t[:], s1[:], mybir.AxisListType.X, ALU.add
                    )
                    mu = small.tile([128, 1], F32, tag="mu")
                    nc.scalar.mul(mu[:], s1t[:], 1.0 / D)
                    mu2 = small.tile([128, 1], F32, tag="mu2")
                    nc.scalar.square(mu2[:], mu[:])
                    var = small.tile([128, 1], F32, tag="var")
                    nc.vector.tensor_scalar(
                        var[:], s2[:], 1.0 / D, mu2[:], op0=ALU.mult, op1=ALU.subtract
                    )
                    std = small.tile([128, 1], F32, tag="std")
                    nc.scalar.activation(std[:], var[:], AF.Sqrt, bias=eps_ln[:])
                    rstd = small.tile([128, 1], F32, tag="rstd")
                    nc.vector.reciprocal(rstd[:], std[:])

                    y = lnt.tile([128, D], BF16, tag="y")
                    yeng = nc.gpsimd if t % 2 == 0 else nc.vector
                    yeng.tensor_scalar(
                        y[:], x_sb[:], mu[:], rstd[:],
                        op0=ALU.subtract, op1=ALU.mult,
                    )
                    if not g_one:
                        nc.vector.tensor_tensor(y[:], y[:], gamma_bc[:], ALU.mult)
                    if not b_zero:
                        nc.vector.tensor_tensor(y[:], y[:], beta_bc[:], ALU.add)
                    nc.sync.dma_start(out[s * 128 : (s + 1) * 128, :], y[:])

    nc.compile()
    return nc


def _get_nc(flags):
    if flags not in _CACHE:
        _CACHE[flags] = _build(*flags)
    return _CACHE[flags]


def _prep(inputs):
    q = np.ascontiguousarray(np.asarray(inputs["query"], dtype=np.float32))
    k = np.ascontiguousarray(np.asarray(inputs["key"], dtype=np.float32))
    v = np.ascontiguousarray(np.asarray(inputs["value"], dtype=np.float32))
    Wq = np.ascontiguousarray(np.asarray(inputs["Wq"], dtype=np.float32))
    Wk = np.ascontiguousarray(np.asarray(inputs["Wk"], dtype=np.float32))
    Wv = np.ascontiguousarray(np.asarray(inputs["Wv"], dtype=np.float32))
    Wo = np.ascontiguousarray(np.asarray(inputs["Wo"], dtype=np.float32))
    bqv = np.ascontiguousarray(np.asarray(inputs["bq"], dtype=np.float32).reshape(1, D))
    bkv = np.ascontiguousarray(np.asarray(inputs["bk"], dtype=np.float32).reshape(1, D))
    bvv = np.ascontiguousarray(np.asarray(inputs["bv"], dtype=np.float32).reshape(1, D))
    bov = np.ascontiguousarray(np.asarray(inputs["bo"], dtype=np.float32).reshape(1, D))
    gv = np.ascontiguousarray(np.asarray(inputs["gamma"], dtype=np.float32).reshape(1, D))
    btv = np.ascontiguousarray(np.asarray(inputs["beta"], dtype=np.float32).reshape(1, D))

    flags = (
        bool(not bqv.any() and not bkv.any()),
        bool(not bvv.any()),
        bool(not bov.any()),
        bool(np.all(gv == 1.0)),
        bool(not btv.any()),
    )
    qf = q.reshape(NCORES, R, D)
    kf = k.reshape(NCORES, R, D)
    vf = v.reshape(NCORES, R, D)
    in_maps = []
    for c in range(NCORES):
        rs = slice(c * 128, (c + 1) * 128)
        in_maps.append(
            {
                "xq": qf[c],
                "xkt": np.ascontiguousarray(kf[c].T),
                "xvt": np.ascontiguousarray(vf[c].T),
                "wq_s": np.ascontiguousarray(Wq[rs]),
                "wk_s": np.ascontiguousarray(Wk[rs]),
                "wv_s": np.ascontiguousarray(Wv[rs]),
                "wo_s": np.ascontiguousarray(Wo[rs]),
                "bq": bqv, "bk": bkv, "bv": bvv, "bo": bov,
                "gamma": gv, "beta": btv,
            }
        )
    return flags, in_maps


def kernel(**inputs):
    from concourse.bass_utils import run_bass_kernel_spmd

    flags, in_maps = _prep(inputs)
    nc = _get_nc(flags)
    res = run_bass_kernel_spmd(nc, in_maps, core_ids=list(range(NCORES)))
    outs = np.stack(
        [np.asarray(res.results[c]["out"], dtype=np.float32) for c in range(NCORES)],
        axis=0,
    )
    return outs.reshape(B, N, D)


# revision 85
# speedup vs baseline: 1.7739x; 1.0201x over previous
"""Trainium2 Bass kernel for linear attention (ELU+1 feature map) block:
Q/K/V projections + linear attention + out-projection + residual + LayerNorm,
distributed over 8 NeuronCores.

Sharding: 8-way row split of (batch*seq); cores 2b, 2b+1 hold the two
2048-token halves of batch b. Per-(batch,head) global reductions (ones-
augmented K^T.Q) are pair-AllReduced. Weights are row-sliced across all 8
cores on the host, cast to fp8 on device, and AllGathered on-chip.

Projections run as fp8e4 DoubleRow matmuls (2 contraction planes per
instruction, 0.5 cyc/row). The feature map is computed as psi = elu(x)
(not elu+1); the ones column of the augmented K matmul reconstructs all
the (psi+1) cross terms after the collective.
"""
import os
import sys

for _p in ("/opt/trn_rl_repo", "/root/.axon_site/_ro/trn_rl_repo"):
    if os.path.isdir(_p) and _p not in sys.path:
        sys.path.insert(0, _p)

import numpy as np

B, N, D, H = 4, 4096, 1024, 16
DEPTH = D // H  # 64
NCORES = 8
R = (B * N) // NCORES  # 2048 rows per core
NSUB = R // 128  # 16 token subtiles per core
NBLK = R // 512  # 4 token blocks in phase B
EPS_Z = 1e-9
EPS_LN = 1e-6
WS = 64.0        # weight pre-scale before fp8 cast
OHS = 16.0       # oh pre-scale before fp8 cast

_CACHE = {}


def _build(zb_qk, zb_v, zb_o, g_one, b_zero, single=False):
    import concourse.bacc as bacc
    import concourse.tile as tile
    from concourse import mybir
    from concourse.masks import make_identity
    from contextlib import ExitStack

    F32 = mybir.dt.float32
    BF16 = mybir.dt.bfloat16
    FP8 = mybir.dt.float8e4
    ALU = mybir.AluOpType
    AF = mybir.ActivationFunctionType
    DR = mybir.MatmulPerfMode.DoubleRow

    nc = bacc.Bacc("TRN2", debug=False, num_devices=1 if single else NCORES)

    xq = nc.dram_tensor("xq", [R, D], F32, kind="ExternalInput").ap()
    xkt = nc.dram_tensor("xkt", [D, R], F32, kind="ExternalInput").ap()
    xvt = nc.dram_tensor("xvt", [D, R], F32, kind="ExternalInput").ap()
    # host-sliced weight rows [c*128:(c+1)*128] of each W
    wq_s = nc.dram_tensor("wq_s", [128, D], F32, kind="ExternalInput").ap()
    wk_s = nc.dram_tensor("wk_s", [128, D], F32, kind="ExternalInput").ap()
    wv_s = nc.dram_tensor("wv_s", [128, D], F32, kind="ExternalInput").ap()
    wo_s = nc.dram_tensor("wo_s", [128, D], F32, kind="ExternalInput").ap()
    bq = nc.dram_tensor("bq", [1, D], F32, kind="ExternalInput").ap()
    bk = nc.dram_tensor("bk", [1, D], F32, kind="ExternalInput").ap()
    bv = nc.dram_tensor("bv", [1, D], F32, kind="ExternalInput").ap()
    bo = nc.dram_tensor("bo", [1, D], F32, kind="ExternalInput").ap()
    gamma = nc.dram_tensor("gamma", [1, D], F32, kind="ExternalInput").ap()
    beta = nc.dram_tensor("beta", [1, D], F32, kind="ExternalInput").ap()
    out = nc.dram_tensor("out", [R, D], BF16, kind="ExternalOutput").ap()

    with tile.TileContext(nc) as tc, ExitStack() as ctx:
        const_p = ctx.enter_context(tc.tile_pool(name="const", bufs=1))
        dp = ctx.enter_context(tc.tile_pool(name="dram", bufs=1, space="DRAM"))
        # long-lived SBUF state
        xq_pool = ctx.enter_context(tc.tile_pool(name="xqn", bufs=1))
        ksb_pool = ctx.enter_context(tc.tile_pool(name="ksb", bufs=1))
        red_pool = ctx.enter_context(tc.tile_pool(name="red", bufs=1))
        wB = ctx.enter_context(tc.tile_pool(name="wB", bufs=1))
        xvt_f_p = ctx.enter_context(tc.tile_pool(name="xvtf", bufs=2))
        xvt_q_p = ctx.enter_context(tc.tile_pool(name="xvtq", bufs=2))

        # ---- weight slice cast + AllGather (first: gates phase A) ----
        cc_w_in = dp.tile([128, 3 * D], FP8, tag="cc_w_in")
        cc_w_out = dp.tile([8 * 128, 3 * D], FP8, tag="cc_w_out")
        cc_wv_in = dp.tile([128, D], BF16, tag="cc_wv_in")
        cc_wv_out = dp.tile([8 * 128, D], BF16, tag="cc_wv_out")
        cc_kq_in = dp.tile([65, D], BF16, tag="cc_kq_in")
        cc_kq_out = dp.tile([65, D], BF16, tag="cc_kq_out")

        s_sel = const_p.tile([16, D], BF16, tag="s_sel")

        from contextlib import ExitStack as _ES0
        with _ES0() as initctx:
            stage = initctx.enter_context(tc.tile_pool(name="stage", bufs=2))
            wslice_q = stage.tile([128, 3 * D], FP8, tag="wslice_q", name="wslice_q")
            for i, w_ap in enumerate((wq_s, wk_s, wo_s)):
                st = stage.tile([128, D], F32, tag="wstage", name=f"wsl_{i}")
                nc.sync.dma_start(st[:], w_ap)
                nc.vector.tensor_scalar_mul(
                    wslice_q[:, i * D : (i + 1) * D], st[:], WS
                )
            nc.sync.dma_start(cc_w_in[:], wslice_q[:])
            # wv slice in bf16 (V path needs the precision; see precsim)
            wv_st = stage.tile([128, D], F32, tag="wstage", name="wsl_v")
            nc.sync.dma_start(wv_st[:], wv_s)
            wv_bf = stage.tile([128, D], BF16, tag="wv_bf", name="wv_bf")
            nc.vector.tensor_copy(wv_bf[:], wv_st[:])
            nc.sync.dma_start(cc_wv_in[:], wv_bf[:])

            if single:
                nc.sync.dma_start(cc_w_out[0:128, :], cc_w_in[:])
                nc.sync.dma_start(cc_wv_out[0:128, :], cc_wv_in[:])
            else:
                nc.gpsimd.collective_compute(
                    "AllGather", ALU.bypass,
                    replica_groups=[list(range(NCORES))],
                    ins=[cc_w_in.opt()], outs=[cc_w_out.opt()],
                )
                nc.gpsimd.collective_compute(
                    "AllGather", ALU.bypass,
                    replica_groups=[list(range(NCORES))],
                    ins=[cc_wv_in.opt()], outs=[cc_wv_out.opt()],
                )

            # S selection matrix (bf16): s_sel[h, f] = 1 iff h == head(f)
            s_f = stage.tile([16, D], F32, tag="wstage", name="s_build",
                             padded_shape=[128, D])
            nc.gpsimd.memset(s_f[:], 0.0)
            s_f3 = s_f[:].rearrange("h (j l) -> h j l", l=64)
            nc.gpsimd.affine_select(
                out=s_f3, in_=s_f3, compare_op=ALU.not_equal, fill=1.0,
                base=0, pattern=[[-1, 16], [0, 64]], channel_multiplier=1,
            )
            nc.vector.tensor_copy(s_sel[:], s_f[:])

        # ---- constants ----
        ident = const_p.tile([128, 128], F32, tag="ident")
        make_identity(nc, ident[:])
        ones_bf = const_p.tile([128, 1], BF16, tag="ones_bf")
        nc.gpsimd.memset(ones_bf[:], 1.0)
        eps_ln = const_p.tile([128, 1], F32, tag="eps_ln")
        nc.gpsimd.memset(eps_ln[:], EPS_LN)

        def bcast_row(name, src_ap):
            row = const_p.tile([1, D], F32, tag=name + "_row")
            nc.sync.dma_start(row[:], src_ap)
            bc = const_p.tile([128, D], F32, tag=name + "_bc")
            nc.gpsimd.partition_broadcast(bc[:], row[:])
            return bc

        bq_bc = None if zb_qk else bcast_row("bq", bq)
        bk_bc = None if zb_qk else bcast_row("bk", bk)
        bo_bc = None if zb_o else bcast_row("bo", bo)
        gamma_bc = None if g_one else bcast_row("gamma", gamma)
        beta_bc = None if b_zero else bcast_row("beta", beta)
        bv_pp = None
        if not zb_v:
            bv_pp = const_p.tile([128, 8], F32, tag="bv_pp")
            for c in range(8):
                nc.sync.dma_start(bv_pp[:, c : c + 1], bv[0:1, c * 128 : (c + 1) * 128])

        # gathered weights -> [128, 8, D] fp8 tiles (k-plane-major pairs)
        cc_w3 = cc_w_out[:].rearrange("(cb p) n -> p cb n", p=128)

        def load_w(pool, widx, name):
            # ACT-queue DMA: waits on the gather without blocking the x
            # prefetch stream on the sync queue
            wt = pool.tile([128, 8, D], FP8, tag=name, name=name)
            nc.scalar.dma_start(wt[:], cc_w3[:, :, widx * D : (widx + 1) * D])
            return wt

        xvt3 = xvt.rearrange("(cb p) m -> p cb m", p=128)

        def load_xvt(blk):
            xvt_q = xvt_q_p.tile([128, 8, 512], BF16, tag="xvt_q",
                                 name=f"xvt_q{blk}")
            for hb in range(4):
                cols = slice(blk * 512 + hb * 128, blk * 512 + (hb + 1) * 128)
                xvt_f = xvt_f_p.tile([128, 8, 128], F32, tag="xvt_f",
                                     name=f"xvt_f{blk}_{hb}")
                nc.sync.dma_start(xvt_f[:], xvt3[:, :, cols])
                nc.gpsimd.tensor_copy(
                    xvt_q[:, :, hb * 128 : (hb + 1) * 128], xvt_f[:]
                )
            return xvt_q

        # ---- persistent activation state ----
        # k_sb[s]: [128 tok, 16 heads, 65] bf16; col 64 = 1.0 (ones augment)
        k_tiles = []
        for s in range(NSUB):
            kt = ksb_pool.tile([128, 16, 65], BF16, tag=f"k_sb{s}", name=f"k_sb{s}")
            nc.gpsimd.memset(kt[:, :, 64:65], 1.0)
            k_tiles.append(kt)
        xq_tiles = [
            xq_pool.tile([128, D], F32, tag=f"xq_nat{s}", name=f"xq_nat{s}")
            for s in range(NSUB)
        ]

        # KtQ rows 0:64 = phiK^T phiQ, row 64 = colsum phiQ (= q_sum)
        ktq_acc = red_pool.tile([65, D], BF16, tag="ktq_acc", padded_shape=[128, D])

        # =========================== PHASE A ===========================
        from contextlib import ExitStack as _ES
        with _ES() as actx:
            wA = actx.enter_context(tc.tile_pool(name="wA", bufs=1))
            wk_t = load_w(wA, 1, "wk_t")
            wq_t = wA.tile([128, 8, D], FP8, tag="wq_t", name="wq_t")
            nc.gpsimd.dma_start(wq_t[:], cc_w3[:, :, 0:D])
            xkt_f_p = actx.enter_context(tc.tile_pool(name="xktf", bufs=2))
            xt_q_p = actx.enter_context(tc.tile_pool(name="xtq", bufs=3))
            qsb_p = actx.enter_context(tc.tile_pool(name="qsb", bufs=3))
            elu_p = actx.enter_context(tc.tile_pool(name="elu", bufs=4))
            psTr = actx.enter_context(tc.tile_pool(name="psTr", bufs=2, space="PSUM"))
            psQK = actx.enter_context(tc.tile_pool(name="psQK", bufs=4, space="PSUM"))
            psKtq = actx.enter_context(tc.tile_pool(name="psKtq", bufs=1, space="PSUM"))

            # KtQ accumulates across all 16 subtiles in 2 resident psum banks:
            # bank0 heads 0-7, bank1 heads 8-15 (cols h*64)
            HB = [(0, 8), (8, 16)]
            kp_banks = []
            for b, (h0, h1) in enumerate(HB):
                kpb = psKtq.tile(
                    [65, (h1 - h0) * 64], F32, tag=f"kp{b}", name=f"kp{b}",
                    padded_shape=[128, (h1 - h0) * 64],
                )
                kp_banks.append(kpb)

            xkt3 = xkt.rearrange("(cb p) m -> p cb m", p=128)

            def project(xt_q, w_t, ps_pool, tag):
                """fp8 DoubleRow projection: out [128 tok, 1024] psum halves."""
                halves = []
                for of in range(2):
                    ph = ps_pool.tile([128, 512], F32, tag="ps_qk", name=f"{tag}_{of}")
                    for u in range(4):
                        nc.tensor.matmul(
                            ph[:],
                            xt_q[:, 2 * u : 2 * u + 2, :],
                            w_t[:, 2 * u : 2 * u + 2, of * 512 : (of + 1) * 512],
                            start=(u == 0), stop=(u == 3),
                            perf_mode=DR,
                        )
                    halves.append(ph)
                return halves

            def phi(halves, dst3, bias_bc, relu_dve):
                """dst3[:, h, 0:64] (bf16) = elu(ps/WS)+1 = relu(ps/WS) + min(e, 1).
                relu on DVE (tensor_scalar) or ACT, to balance the engines."""
                for of in range(2):
                    src = halves[of][:]
                    if bias_bc is not None:
                        xb = elu_p.tile([128, 512], F32, tag="xb")
                        sl = slice(of * 512, (of + 1) * 512)
                        nc.vector.scalar_tensor_tensor(
                            xb[:], src, 1.0 / WS, bias_bc[:, sl],
                            op0=ALU.mult, op1=ALU.add,
                        )
                        src = xb[:]
                        scl = 1.0
                    else:
                        scl = 1.0 / WS
                    e = elu_p.tile([128, 512], BF16, tag="e")
                    nc.scalar.activation(e[:], src, AF.Exp, scale=scl)
                    r = elu_p.tile([128, 512], BF16, tag="r")
                    if relu_dve:
                        nc.vector.tensor_scalar(
                            r[:], src, scl, 0.0, op0=ALU.mult, op1=ALU.max
                        )
                    else:
                        nc.scalar.activation(r[:], src, AF.Relu, scale=scl)
                    dst = dst3[:, of * 8 : (of + 1) * 8, 0:64]
                    nc.vector.scalar_tensor_tensor(
                        dst, e[:], 1.0, r[:], op0=ALU.min, op1=ALU.add
                    )

            for s in range(NSUB):
                rows = slice(s * 128, (s + 1) * 128)
                xq_nat = xq_tiles[s]
                nc.sync.dma_start(xq_nat[:], xq[rows, :])
                xkt_f = xkt_f_p.tile([128, 8, 128], F32, tag="xkt_f")
                nc.sync.dma_start(xkt_f[:], xkt3[:, :, rows])
                xkt_q = xt_q_p.tile([128, 8, 128], FP8, tag="xkt_q")
                nc.gpsimd.tensor_copy(xkt_q[:], xkt_f[:])

                # transpose xq -> fp8 xqt (2 psum banks, 4 transposes each);
                # drains split across ACT/DVE
                xqt_q = xt_q_p.tile([128, 8, 128], FP8, tag="xqt_q")
                for g in range(2):
                    ps_t = psTr.tile([128, 512], F32, tag="trA")
                    for j in range(4):
                        c = g * 4 + j
                        nc.tensor.transpose(
                            ps_t[:, j * 128 : (j + 1) * 128],
                            xq_nat[:, c * 128 : (c + 1) * 128],
                            ident[:],
                        )
                    if g == 0:
                        nc.scalar.copy(xqt_q[:, 0:4, :], ps_t[:])
                    else:
                        nc.vector.tensor_copy(xqt_q[:, 4:8, :], ps_t[:])

                ps_k = project(xkt_q, wk_t, psQK, "ps_k")
                phi(ps_k, k_tiles[s][:], bk_bc, relu_dve=False)

                ps_q = project(xqt_q, wq_t, psQK, "ps_q")
                q_sb = qsb_p.tile([128, 16, 64], BF16, tag="q_sb")
                phi(ps_q, q_sb[:], bq_bc, relu_dve=True)

                # ones-augmented KtQ accumulated in resident psum banks:
                # out[d, (h, e)]: rows 0:64 phiK^T phiQ, row 64 colsum phiQ
                for b, (h0, h1) in enumerate(HB):
                    for h in range(h0, h1):
                        off = (h - h0) * 64
                        nc.tensor.matmul(
                            kp_banks[b][:, off : off + 64],
                            k_tiles[s][:, h, :],
                            q_sb[:, h, :],
                            start=(s == 0), stop=(s == NSUB - 1),
                        )

            # drain the resident KtQ psum banks
            for b, (h0, h1) in enumerate(HB):
                nc.scalar.copy(
                    ktq_acc[:, h0 * 64 : h1 * 64], kp_banks[b][:]
                )

            # prefetch V inputs for the first two phase-B blocks
            wv_t = wB.tile([128, 8, D], BF16, tag="wv_t", name="wv_t")
            nc.scalar.dma_start(
                wv_t[:], cc_wv_out[:].rearrange("(cb p) n -> p cb n", p=128)
            )
            wo_t = load_w(wB, 2, "wo_t")
            xvt_pre = {blk: load_xvt(blk) for blk in range(2)}

        nc.sync.dma_start(cc_kq_in[:], ktq_acc[:])
        if single:
            nc.sync.dma_start(cc_kq_out[:], cc_kq_in[:])
        else:
            nc.gpsimd.collective_compute(
                "AllReduce", ALU.add,
                replica_groups=[[0, 1], [2, 3], [4, 5], [6, 7]],
                ins=[cc_kq_in.opt()], outs=[cc_kq_out.opt()],
            )

        # =========================== PHASE B ===========================
        with _ES() as bctx:
            bred = bctx.enter_context(tc.tile_pool(name="bred", bufs=1))
            vsb_p = bctx.enter_context(tc.tile_pool(name="vsb", bufs=2))
            oh_p = bctx.enter_context(tc.tile_pool(name="oh", bufs=2))
            zt_p = bctx.enter_context(tc.tile_pool(name="zt", bufs=3))
            div_p = bctx.enter_context(tc.tile_pool(name="divsb", bufs=2))
            lnt = bctx.enter_context(tc.tile_pool(name="lnt", bufs=2))
            lns = bctx.enter_context(tc.tile_pool(name="lns", bufs=1))
            small = bctx.enter_context(tc.tile_pool(name="small", bufs=4))
            psV = bctx.enter_context(tc.tile_pool(name="psV", bufs=2, space="PSUM"))
            psNum = bctx.enter_context(tc.tile_pool(name="psNum", bufs=2, space="PSUM"))
            psDiv = bctx.enter_context(tc.tile_pool(name="psDiv", bufs=1, space="PSUM"))
            psO = bctx.enter_context(tc.tile_pool(name="psO", bufs=2, space="PSUM"))
            psIz = bctx.enter_context(tc.tile_pool(name="psIz", bufs=1, space="PSUM"))

            def phase_b_setup():
                """aug rows 0:64 are KtQ (phi cross-products), row 64 is q_sum.
                Build the block-diagonal bf16 lhsT and the q_sum broadcast."""
                qsum_bc = bred.tile([128, D], BF16, tag="qsum_bc")
                ktq_r = bred.tile([128, 8, 128], BF16, tag="ktq_r")
                with _ES() as sctx:
                    tmp = sctx.enter_context(tc.tile_pool(name="pbtmp", bufs=1))
                    rq_bf = tmp.tile([1, D], BF16, tag="rq_bf", padded_shape=[128, D])
                    nc.sync.dma_start(rq_bf[:], cc_kq_out[64:65, :])
                    nc.gpsimd.partition_broadcast(qsum_bc[:], rq_bf[:])

                    aug_bf = tmp.tile([64, D], BF16, tag="aug_bf",
                                      padded_shape=[128, D])
                    nc.sync.dma_start(aug_bf[:], cc_kq_out[0:64, :])
                    nc.gpsimd.memset(ktq_r[:], 0.0)
                    ktv = aug_bf[:].rearrange("d (c t e) -> d c t e", t=2, e=64)
                    nc.sync.dma_start(ktq_r[0:64, :, 0:64], ktv[:, :, 0, :])
                    nc.sync.dma_start(ktq_r[64:128, :, 64:128], ktv[:, :, 1, :])
                return ktq_r, qsum_bc

            ktq_r = qsum_bc = None

            for blk in range(NBLK):
                xvt_q = xvt_pre.pop(blk, None) or load_xvt(blk)
                if blk + 2 < NBLK:
                    xvt_pre[blk + 2] = load_xvt(blk + 2)

                if blk == 0:
                    ktq_r, qsum_bc = phase_b_setup()

                # ---- V projection (feature-major), bf16 for precision ----
                v_sb = vsb_p.tile([128, 8, 512], BF16, tag="v_sb")
                for c in range(8):
                    ps_v = psV.tile([128, 512], F32, tag="ps_v")
                    for u in range(8):
                        nc.tensor.matmul(
                            ps_v[:],
                            wv_t[:, u, c * 128 : (c + 1) * 128],
                            xvt_q[:, u, :],
                            start=(u == 0), stop=(u == 7),
                        )
                    if zb_v:
                        nc.scalar.copy(v_sb[:, c, :], ps_v[:])
                    else:
                        nc.scalar.activation(
                            v_sb[:, c, :], ps_v[:], AF.Identity,
                            bias=bv_pp[:, c : c + 1],
                        )

                # ---- z = psiK . qsum_phi + S (token-major), invz -> feat-major
                invz_fm = div_p.tile([16, 512], BF16, tag="invz_fm",
                                     padded_shape=[128, 512])
                for t in range(4):
                    s = blk * 4 + t
                    prod = zt_p.tile([128, 16, 64], BF16, tag="prod")
                    nc.vector.tensor_tensor(
                        prod[:], k_tiles[s][:, :, 0:64],
                        qsum_bc[:].rearrange("p (h e) -> p h e", e=64),
                        ALU.mult,
                    )
                    zraw = zt_p.tile([128, 16], F32, tag="zraw")
                    nc.vector.tensor_reduce(
                        zraw[:], prod[:], mybir.AxisListType.X, ALU.add
                    )
                    z2 = zt_p.tile([128, 16], F32, tag="z2")
                    nc.vector.tensor_scalar_add(z2[:], zraw[:], EPS_Z)
                    iz = zt_p.tile([128, 16], F32, tag="iz")
                    nc.vector.reciprocal(iz[:], z2[:])
                    ps_zt = psIz.tile([16, 128], F32, tag="ps_zt",
                                      padded_shape=[128, 128])
                    nc.tensor.transpose(ps_zt[:], iz[:], ident[:])
                    nc.scalar.copy(invz_fm[:, t * 128 : (t + 1) * 128], ps_zt[:])

                # ---- numerator + divisor + oh (feature-major, fp8) ----
                oh_all = oh_p.tile([128, 8, 512], FP8, tag="oh_all")
                for c in range(8):
                    ps_n = psNum.tile([128, 512], F32, tag="ps_n")
                    nc.tensor.matmul(
                        ps_n[:], ktq_r[:, c, :], v_sb[:, c, :],
                        start=True, stop=True,
                    )
                    ps_d = psDiv.tile([128, 512], F32, tag="ps_d")
                    nc.tensor.matmul(
                        ps_d[:], s_sel[:, c * 128 : (c + 1) * 128], invz_fm[:],
                        start=True, stop=True,
                    )
                    num_sb = div_p.tile([128, 512], BF16, tag="num_sb")
                    nc.scalar.mul(num_sb[:], ps_n[:], OHS)
                    nc.vector.scalar_tensor_tensor(
                        oh_all[:, c, :], ps_d[:], 1.0, num_sb[:],
                        op0=ALU.mult, op1=ALU.mult,
                    )

                # ---- out-projection + residual + LayerNorm ----
                for t in range(4):
                    s = blk * 4 + t
                    res = xq_tiles[s][:]
                    if bo_bc is not None:
                        qb = lnt.tile([128, D], F32, tag="qb")
                        nc.vector.tensor_tensor(qb[:], res, bo_bc[:], ALU.add)
                        res = qb[:]

                    x_sb = lnt.tile([128, D], BF16, tag="x_sb")
                    s1 = small.tile([128, 2], F32, tag="s1")
                    for of in range(2):
                        sl = slice(of * 512, (of + 1) * 512)
                        ps_o = psO.tile([128, 512], F32, tag="ps_o")
                        for u in range(4):
                            nc.tensor.matmul(
                                ps_o[:],
                                oh_all[:, 2 * u : 2 * u + 2, t * 128 : (t + 1) * 128],
                                wo_t[:, 2 * u : 2 * u + 2, sl],
                                start=(u == 0), stop=(u == 3),
                                perf_mode=DR,
                            )
                        nc.vector.scalar_tensor_tensor(
                            x_sb[:, sl], ps_o[:], 1.0 / (WS * OHS), res[:, sl],
                            op0=ALU.mult, op1=ALU.add,
                            accum_out=s1[:, of : of + 1],
                        )
                    # LN stats: sum via ACT square-accum, mean via matmul-free ops
                    sq = lns.tile([128, D], BF16, tag="sq")
                    s2 = small.tile([128, 1], F32, tag="s2")
                    nc.scalar.activation(sq[:], x_sb[:], AF.Square, accum_out=s2[:])
                    s1t = small.tile([128, 1], F32, tag="s1t")
                    nc.vector.tensor_reduce(
                        s1t[:], s1[:], mybir.AxisListType.X, ALU.add
                    )
                    mu = small.tile([128, 1], F32, tag="mu")
                    nc.scalar.mul(mu[:], s1t[:], 1.0 / D)
                    mu2 = small.tile([128, 1], F32, tag="mu2")
                    nc.scalar.square(mu2[:], mu[:])
                    var = small.tile([128, 1], F32, tag="var")
                    nc.vector.tensor_scalar(
                        var[:], s2[:], 1.0 / D, mu2[:], op0=ALU.mult, op1=ALU.subtract
                    )
                    std = small.tile([128, 1], F32, tag="std")
                    nc.scalar.activation(std[:], var[:], AF.Sqrt, bias=eps_ln[:])
                    rstd = small.tile([128, 1], F32, tag="rstd")
                    nc.vector.reciprocal(rstd[:], std[:])

                    y = lnt.tile([128, D], BF16, tag="y")
                    yeng = nc.gpsimd if t % 2 == 0 else nc.vector
                    yeng.tensor_scalar(
                        y[:], x_sb[:], mu[:], rstd[:],
                        op0=ALU.subtract, op1=ALU.mult,
                    )
                    if not g_one:
                        nc.vector.tensor_tensor(y[:], y[:], gamma_bc[:], ALU.mult)
                    if not b_zero:
                        nc.vector.tensor_tensor(y[:], y[:], beta_bc[:], ALU.add)
                    nc.sync.dma_start(out[s * 128 : (s + 1) * 128, :], y[:])

    nc.compile()
    return nc


def _get_nc(flags):
    if flags not in _CACHE:
        _CACHE[flags] = _build(*flags)
    return _CACHE[flags]


def _prep(inputs):
    q = np.ascontiguousarray(np.asarray(inputs["query"], dtype=np.float32))
    k = np.ascontiguousarray(np.asarray(inputs["key"], dtype=np.float32))
    v = np.ascontiguousarray(np.asarray(inputs["value"], dtype=np.float32))
    Wq = np.ascontiguousarray(np.asarray(inputs["Wq"], dtype=np.float32))
    Wk = np.ascontiguousarray(np.asarray(inputs["Wk"], dtype=np.float32))
    Wv = np.ascontiguousarray(np.asarray(inputs["Wv"], dtype=np.float32))
    Wo = np.ascontiguousarray(np.asarray(inputs["Wo"], dtype=np.float32))
    bqv = np.ascontiguousarray(np.asarray(inputs["bq"], dtype=np.float32).reshape(1, D))
    bkv = np.ascontiguousarray(np.asarray(inputs["bk"], dtype=np.float32).reshape(1, D))
    bvv = np.ascontiguousarray(np.asarray(inputs["bv"], dtype=np.float32).reshape(1, D))
    bov = np.ascontiguousarray(np.asarray(inputs["bo"], dtype=np.float32).reshape(1, D))
    gv = np.ascontiguousarray(np.asarray(inputs["gamma"], dtype=np.float32).reshape(1, D))
    btv = np.ascontiguousarray(np.asarray(inputs["beta"], dtype=np.float32).reshape(1, D))

    flags = (
        bool(not bqv.any() and not bkv.any()),
        bool(not bvv.any()),
        bool(not bov.any()),
        bool(np.all(gv == 1.0)),
        bool(not btv.any()),
    )
    qf = q.reshape(NCORES, R, D)
    kf = k.reshape(NCORES, R, D)
    vf = v.reshape(NCORES, R, D)
    in_maps = []
    for c in range(NCORES):
        rs = slice(c * 128, (c + 1) * 128)
        in_maps.append(
            {
                "xq": qf[c],
                "xkt": np.ascontiguousarray(kf[c].T),
                "xvt": np.ascontiguousarray(vf[c].T),
                "wq_s": np.ascontiguousarray(Wq[rs]),
                "wk_s": np.ascontiguousarray(Wk[rs]),
                "wv_s": np.ascontiguousarray(Wv[rs]),
                "wo_s": np.ascontiguousarray(Wo[rs]),
                "bq": bqv, "bk": bkv, "bv": bvv, "bo": bov,
                "gamma": gv, "beta": btv,
            }
        )
    return flags, in_maps


def kernel(**inputs):
    from concourse.bass_utils import run_bass_kernel_spmd

    flags, in_maps = _prep(inputs)
    nc = _get_nc(flags)
    res = run_bass_kernel_spmd(nc, in_maps, core_ids=list(range(NCORES)))
    outs = np.stack(
        [np.asarray(res.results[c]["out"], dtype=np.float32) for c in range(NCORES)],
        axis=0,
    )
    return outs.reshape(B, N, D)
